# revision 1
# baseline (speedup 1.0000x reference)
"""Trainium2 Bass kernel for nn_Complex_Only_46308337385506 (gnn_message_passing).

Math (derived + numerically validated against the jax reference):
  The per-edge orthonormal basis R (rows nU, nV, nJ) enters the output only
  through two per-edge scalars:
      gam = nJ_z = Jz/(|J|+eps)
      A1p = copysign(sqrt(Jx^2+Jy^2), gam+eps)/(|J|+eps)    (= -nU_z approx)
  With w = gam*Xz - A1p*Xx:
      Y0 = Wa@Xx + (Wa-Wc)@(A1p*w) + Wb@(gam*Xy)
      Y1 = Wa@Xy - Wb@(A1p*Xz + gam*Xx)
      Y2 = Wa@Xz + (Wc-Wa)@(gam*w) + Wb@(A1p*Xy)
  followed by the VN leaky-relu stage:
      d = Wd@Y (over channel dim), dot = <Y,d>_3, dn2 = <d,d>_3
      out = Y - 0.8*min(dot,0)/(dn2+eps) * d

Sharding: data-parallel over batch B=8 -> one batch per NeuronCore.
Per-core layout: supers of 1024 points; points are transposed on the PE
(pairs of feature blocks) so the E-contraction runs as [K<=128, N=512]
matmuls; stage-3 runs on [128, 512] tiles (two 512-pt groups stacked on
partitions).

Perf notes (cost-model 246us -> 196us):
  - All contraction matmuls run in fp32r (1 cycle/row vs 4 for fp32 at
    N>=512). The HW verifier requires fp32r operands to be *written* as
    fp32r, so the weight stack is rounded once via an ACT copy (wsr) and
    the rh/xsb tiles are declared float32r (their PSUM->SBUF producer
    copies do the rounding). Transposes stay fp32 (PE has slack).
  - Elementwise/copy work is balanced across ACT/DVE/Pool by cost-model
    price (612/594-658/427 ns per [128,512] op) under the HW constraint
    that GPSIMD may not touch PSUM: PSUM->SBUF copies live on ACT+DVE
    only; SBUF-only tensor ops are biased onto Pool.
  - X/J are loaded per-super (not per-2-supers), halving the io pool so
    the VN-stage pool (s3p) can double-buffer across supers.
"""

import math
import os
import numpy as np
from contextlib import ExitStack

import concourse.bass as bass
import concourse.bacc as bacc
import concourse.tile as tile
from concourse import mybir
from concourse import bass_utils

F32 = mybir.dt.float32
F32R = mybir.dt.float32r
U32 = mybir.dt.uint32
AF = mybir.ActivationFunctionType
ALU = mybir.AluOpType

EPS = 1e-6
NEG = 0.2

B, C, E = 8, 16384, 64
SUPER = 1024           # points per super-iteration
NSUP = C // SUPER      # 16
GROUP = 512            # matmul free dim (points)
NCHUNK = 8             # 128-pt chunks per super


_CUSTOM_OPS = {}


def _register_custom_dve_ops():
    """Register two fused DVE ops (module-level, idempotent):
      SQSUM_ANT: out = Src0^2 + Src1^2
      ADDSQ_ANT: out = Src0 + Src1^2
    Replaces {2x ACT Square + 1 DVE add} chains with one DVE pass each."""
    if _CUSTOM_OPS:
        return _CUSTOM_OPS
    import numpy as _np
    from concourse import dve_ops
    from concourse.dve_spec import Spec, Src0, Src1, lower, sq, _has_src1
    from concourse.dve_uop import DveOpSpec
    from concourse.dve_table_gen import dve_ver_for

    def make(name, body, ref):
        spec = Spec(body=body, reference=ref)
        opcode = dve_ops._CUSTOM_DVE_ROW_BASE + len(dve_ops.OPS)
        shas = {}
        for ver in ("v3", "v4"):
            try:
                s = DveOpSpec(name=name, opcode=opcode,
                              uops=lower(spec, ver=ver),
                              rd1_en=_has_src1(spec))
                shas[ver] = s.sha(ver)
            except Exception:
                pass
        op = dve_ops.DveOp(name, spec, subdim=False, uops_sha=shas)
        dve_ops.OPS.append(op)
        dve_ops.CUSTOM_DVE_SPECS[name] = spec
        dve_ops._SUB_OPCODE_FOR_NAME[name] = opcode
        assert opcode < 0x20
        return op

    _CUSTOM_OPS["SQSUM"] = make(
        "SQSUM_ANT", sq(Src0) + sq(Src1),
        lambda in0, in1, s0, s1, imm2:
            (in0.astype(_np.float32) * in0 + in1.astype(_np.float32) * in1))
    _CUSTOM_OPS["ADDSQ"] = make(
        "ADDSQ_ANT", Src0 + sq(Src1),
        lambda in0, in1, s0, s1, imm2:
            in0.astype(_np.float32) + in1.astype(_np.float32) * in1)
    from concourse.dve_spec import C0
    _CUSTOM_OPS["ADDSQS"] = make(
        "ADDSQS_ANT", (Src0 + sq(Src1)) * C0,
        lambda in0, in1, s0, s1, imm2:
            (in0.astype(_np.float32) + in1.astype(_np.float32) * in1) * s0)
    return _CUSTOM_OPS


def _pin_act_table_set(arch: str):
    """Steer the ACT table-set chooser: all funcs this kernel uses must
    first-match natural_log_exp_and_others, so exactly one table load is
    emitted (the chooser first-matches in act_info.json order)."""
    from concourse import hw_specs
    tables = hw_specs.get_activation_tables(arch)  # cached dict, mutate in place
    mine = {AF.Ln, AF.Exp, AF.Square, AF.Copy, AF.Identity}
    for name, funcs in tables.items():
        if name != "natural_log_exp_and_others":
            funcs -= mine


def _build_nc():
    global OPS
    OPS = _register_custom_dve_ops()
    nc = bacc.Bacc("TRN2", debug=False)
    _pin_act_table_set(nc.m.arch)

    XS = nc.dram_tensor("XS", [C, 192], F32, kind="ExternalInput").ap()
    JS = nc.dram_tensor("JS", [C, 192], F32, kind="ExternalInput").ap()
    WMM = nc.dram_tensor("WMM", [6, 128, 128], F32, kind="ExternalInput").ap()
    OUT = nc.dram_tensor("OUT", [64, 3, C], F32, kind="ExternalOutput").ap()

    with tile.TileContext(nc) as tc, ExitStack() as ctx:
        const = ctx.enter_context(tc.tile_pool(name="const", bufs=1))
        io = ctx.enter_context(tc.tile_pool(name="io", bufs=2))
        sa = ctx.enter_context(tc.tile_pool(name="sa", bufs=1))
        prodp = ctx.enter_context(tc.tile_pool(name="prodp", bufs=2))
        rhsp = ctx.enter_context(tc.tile_pool(name="rhsp", bufs=2))
        xsbp = ctx.enter_context(tc.tile_pool(name="xsbp", bufs=2))
        s3p = ctx.enter_context(tc.tile_pool(name="s3p", bufs=2))
        outp = ctx.enter_context(tc.tile_pool(name="outp", bufs=2))
        psT = ctx.enter_context(tc.tile_pool(name="psT", bufs=1, space="PSUM"))
        psY = ctx.enter_context(tc.tile_pool(name="psY", bufs=1, space="PSUM"))
        psD = ctx.enter_context(tc.tile_pool(name="psD", bufs=2, space="PSUM"))

        # bias constants for ACT
        eps_c = const.tile([128, 1], F32, tag="eps_c")
        ln8_c = const.tile([128, 1], F32, tag="ln8_c")
        nc.gpsimd.memset(eps_c[:], EPS)
        nc.gpsimd.memset(ln8_c[:], float(math.log(1.0 - NEG)))
        sgn_c = const.tile([128, 1], U32, tag="sgn_c")
        nc.gpsimd.memset(sgn_c[:], 0x80000000)

        # weights + identity, loaded once
        wsb = const.tile([128, 6, 128], F32)
        nc.sync.dma_start(wsb[:], WMM.rearrange("n p m -> p n m"))
        # fp32r stationaries: PE runs fp32r matmuls at 1 cycle/row (vs 4 for
        # fp32) for N>=256. The verifier requires f32r operands to be written
        # as f32r, so round the weight stack once via an ACT copy.
        wsr = const.tile([128, 5, 128], F32R)
        nc.scalar.activation(wsr[:], wsb[:, 0:5, :], AF.Copy)
        IDT = wsb[:, 5, :]       # identity (fp32 transposes)
        LW_A = wsr[:, 0, :]      # blkdiag(WaT, WaT)
        LW_2 = wsr[:, 1, :]      # blkdiag((Wa-Wc).T, (Wc-Wa).T)
        LW_B = wsr[:, 2, :]      # blkdiag(WbT, WbT)
        LW_1 = wsr[:, 3, 0:64]   # [WaT; -WbT], M=64
        LW_D = wsr[:, 4, :]      # blkdiag(WdT, WdT)

        X3 = XS.rearrange("(v s p) w -> v p s w", p=128, s=NCHUNK)
        J3 = JS.rearrange("(v s p) w -> v p s w", p=128, s=NCHUNK)

        for u in range(NSUP):
            xst = io.tile([128, NCHUNK * 192], F32, tag="xs")
            jst = io.tile([128, NCHUNK * 192], F32, tag="js")
            nc.sync.dma_start(
                xst[:].rearrange("p (s w) -> p s w", s=NCHUNK, w=192), X3[u])
            nc.sync.dma_start(
                jst[:].rearrange("p (s w) -> p s w", s=NCHUNK, w=192), J3[u])
            xs = xst[:]
            js = jst[:]
            xv = xs.rearrange("p (s e c) -> p s e c", s=NCHUNK, e=E, c=3)
            jv = js.rearrange("p (s e c) -> p s e c", s=NCHUNK, e=E, c=3)

            def v3(t):  # [128, 512] tile -> [128, 8, 64] view
                return t[:].rearrange("p (s e) -> p s e", s=NCHUNK, e=E)

            # ---- stage A: per-edge scalars gam, A1p --------------------
            q = sa.tile([128, SUPER // 2], F32, tag="q")
            n2 = sa.tile([128, SUPER // 2], F32, tag="n2")
            nc.vector._custom_dve(OPS["SQSUM"], out=v3(q),
                                  in0=jv[:, :, :, 0], in1=jv[:, :, :, 1])
            nc.vector._custom_dve(OPS["ADDSQ"], out=v3(n2),
                                  in0=v3(q), in1=jv[:, :, :, 2])
            # ln-domain: t = rsqrt(n2) = exp(-0.5 ln n2);
            # |A1| = sqrt(q/n2) = exp(0.5 (ln q - ln n2)); sign from gam+eps.
            # (dropping the +EPS inside t shifts gam by ~1e-6 rel: negligible)
            lq = sa.tile([128, SUPER // 2], F32, tag="lq")
            ln2 = sa.tile([128, SUPER // 2], F32, tag="ln2")
            nc.scalar.activation(lq[:], q[:], AF.Ln)
            nc.scalar.activation(ln2[:], n2[:], AF.Ln)
            t_ = sa.tile([128, SUPER // 2], F32, tag="t_")
            nc.scalar.activation(t_[:], ln2[:], AF.Exp, scale=-0.5)
            df = sa.tile([128, SUPER // 2], F32, tag="df")
            nc.gpsimd.tensor_tensor(df[:], lq[:], ln2[:], ALU.subtract)
            rho = sa.tile([128, SUPER // 2], F32, tag="rho")
            nc.scalar.activation(rho[:], df[:], AF.Exp, scale=0.5)
            gam = sa.tile([128, SUPER // 2], F32, tag="gam")
            nc.gpsimd.tensor_tensor(v3(gam), jv[:, :, :, 2], v3(t_), ALU.mult)
            h = sa.tile([128, SUPER // 2], F32, tag="h")
            nc.vector.tensor_scalar(h[:], gam[:], EPS, None, ALU.add)
            a1 = sa.tile([128, SUPER // 2], F32, tag="a1")
            nc.vector.scalar_tensor_tensor(
                a1[:].bitcast(U32), h[:].bitcast(U32), sgn_c[:],
                rho[:].bitcast(U32), ALU.bitwise_and, ALU.bitwise_or)

            # ---- stage B: feature blocks into PROD slots ---------------
            # slots: 0=Xx 1=Xz 2=A1p*w 3=gam*w 4=gam*Xy 5=A1p*Xy 6=Xy 7=c8
            prod = prodp.tile([128, NCHUNK, 8, E], F32, tag="prod")
            nc.gpsimd.tensor_copy(prod[:, :, 0, :], xv[:, :, :, 0])
            nc.gpsimd.tensor_copy(prod[:, :, 1, :], xv[:, :, :, 2])
            nc.gpsimd.tensor_copy(prod[:, :, 6, :], xv[:, :, :, 1])
            m1 = sa.tile([128, SUPER // 2], F32, tag="m1")
            m2 = sa.tile([128, SUPER // 2], F32, tag="m2")
            wt = sa.tile([128, SUPER // 2], F32, tag="wt")
            nc.gpsimd.tensor_tensor(v3(m1), v3(gam), xv[:, :, :, 2], ALU.mult)
            nc.gpsimd.tensor_tensor(v3(m2), v3(a1), xv[:, :, :, 0], ALU.mult)
            nc.gpsimd.tensor_tensor(wt[:], m1[:], m2[:], ALU.subtract)
            nc.gpsimd.tensor_tensor(prod[:, :, 2, :], v3(a1), v3(wt), ALU.mult)
            nc.gpsimd.tensor_tensor(prod[:, :, 3, :], v3(gam), v3(wt), ALU.mult)
            nc.gpsimd.tensor_tensor(prod[:, :, 4, :], v3(gam), xv[:, :, :, 1],
                                    ALU.mult)
            nc.gpsimd.tensor_tensor(prod[:, :, 5, :], v3(a1), xv[:, :, :, 1],
                                    ALU.mult)
            m3 = sa.tile([128, SUPER // 2], F32, tag="m3")
            m4 = sa.tile([128, SUPER // 2], F32, tag="m4")
            nc.gpsimd.tensor_tensor(v3(m3), v3(a1), xv[:, :, :, 2], ALU.mult)
            nc.gpsimd.tensor_tensor(v3(m4), v3(gam), xv[:, :, :, 0], ALU.mult)
            nc.gpsimd.tensor_tensor(prod[:, :, 7, :], v3(m3), v3(m4), ALU.add)

            # ---- per group: transpose, matmuls, Y copies ---------------
            xsb = xsbp.tile([128, 3, GROUP], F32R, tag="xsb")
            for g in range(2):
                tpX = psT.tile([128, GROUP], F32, tag="tpX")
                tp1 = psT.tile([128, GROUP], F32, tag="tp1")
                tp2 = psT.tile([128, GROUP], F32, tag="tp2")
                tp3 = psT.tile([128, GROUP], F32, tag="tp3")
                for k in range(4):
                    s = 4 * g + k
                    sl = slice(128 * k, 128 * (k + 1))
                    nc.tensor.transpose(tpX[:, sl], prod[:, s, 0:2, :], IDT)
                    nc.tensor.transpose(tp1[:, sl], prod[:, s, 2:4, :], IDT)
                    nc.tensor.transpose(tp2[:, sl], prod[:, s, 4:6, :], IDT)
                    nc.tensor.transpose(tp3[:, sl], prod[:, s, 6:8, :], IDT)
                rhX = rhsp.tile([128, GROUP], F32R, tag="rhX")
                rh1 = rhsp.tile([128, GROUP], F32R, tag="rh1")
                rh2 = rhsp.tile([128, GROUP], F32R, tag="rh2")
                rh3 = rhsp.tile([128, GROUP], F32R, tag="rh3")
                nc.scalar.activation(rhX[:], tpX[:], AF.Copy)
                nc.scalar.activation(rh1[:], tp1[:], AF.Copy)
                nc.scalar.activation(rh2[:], tp2[:], AF.Copy)
                nc.scalar.activation(rh3[:], tp3[:], AF.Copy)

                pA = psY.tile([128, GROUP], F32, tag="pA")
                pB = psY.tile([64, GROUP], F32, tag="pB")
                nc.tensor.matmul(pA[:], LW_A, rhX[:], start=True, stop=False)
                nc.tensor.matmul(pA[:], LW_2, rh1[:], start=False, stop=False)
                nc.tensor.matmul(pA[:], LW_B, rh2[:], start=False, stop=True)
                nc.tensor.matmul(pB[:], LW_1, rh3[:], start=True, stop=True)

                ro = slice(64 * g, 64 * (g + 1))
                nc.vector.tensor_copy(xsb[ro, 0, :], pA[0:64, :])
                nc.vector.tensor_copy(xsb[ro, 2, :], pA[64:128, :])
                if g == 0:
                    nc.scalar.activation(xsb[ro, 1, :], pB[:], AF.Copy)
                else:
                    nc.vector.tensor_copy(xsb[ro, 1, :], pB[:])

            # ---- Wd stage + VN leaky relu ------------------------------
            dsb = s3p.tile([128, 3, GROUP], F32, tag="dsb")
            for i in range(3):
                pd = psD.tile([128, GROUP], F32, tag="pd")
                nc.tensor.matmul(pd[:], LW_D, xsb[:, i, :], start=True,
                                 stop=True)
                nc.scalar.activation(dsb[:, i, :], pd[:], AF.Copy)

            xd0 = s3p.tile([128, GROUP], F32, tag="xd0")
            xd1 = s3p.tile([128, GROUP], F32, tag="xd1")
            xd2 = s3p.tile([128, GROUP], F32, tag="xd2")
            dot = s3p.tile([128, GROUP], F32, tag="dot")
            nc.gpsimd.tensor_tensor(xd0[:], xsb[:, 0, :].bitcast(F32), dsb[:, 0, :],
                                    ALU.mult)
            nc.gpsimd.tensor_tensor(xd1[:], xsb[:, 1, :].bitcast(F32), dsb[:, 1, :],
                                    ALU.mult)
            nc.gpsimd.tensor_tensor(xd2[:], xsb[:, 2, :].bitcast(F32), dsb[:, 2, :],
                                    ALU.mult)
            nc.gpsimd.tensor_tensor(dot[:], xd0[:], xd1[:], ALU.add)
            nc.vector.tensor_tensor(dot[:], dot[:], xd2[:], ALU.add)

            dn2 = s3p.tile([128, GROUP], F32, tag="dn2")
            nc.vector._custom_dve(OPS["SQSUM"], out=dn2[:],
                                  in0=dsb[:, 0, :], in1=dsb[:, 1, :])
            nc.vector._custom_dve(OPS["ADDSQS"], out=dn2[:],
                                  in0=dn2[:], in1=dsb[:, 2, :],
                                  s0=1.0 / (1.0 - NEG))

            rec = s3p.tile([128, GROUP], F32, tag="rec")
            nc.vector.reciprocal_approx_fast(rec[:], dn2[:])
            s2 = s3p.tile([128, GROUP], F32, tag="s2")
            nc.vector.scalar_tensor_tensor(s2[:], dot[:], 0.0, rec[:],
                                           ALU.min, ALU.mult)

            ot = outp.tile([128, 3, GROUP], F32, tag="ot")
            for i in range(3):
                mi = s3p.tile([128, GROUP], F32, tag=f"mi{i}")
                nc.gpsimd.tensor_tensor(mi[:], s2[:], dsb[:, i, :], ALU.mult)
                eng = nc.gpsimd if i == 0 else nc.vector
                eng.tensor_tensor(ot[:, i, :], xsb[:, i, :].bitcast(F32),
                                  mi[:], ALU.subtract)

            c0 = u * SUPER
            nc.sync.dma_start(OUT[:, :, c0:c0 + GROUP], ot[0:64])
            nc.sync.dma_start(OUT[:, :, c0 + GROUP:c0 + SUPER], ot[64:128])

    nc.compile()
    return nc


_NC = None


def _get_nc():
    global _NC
    if _NC is None:
        _NC = _build_nc()
    return _NC


def _weight_stack(Wa, Wb, Wc, Wd):
    Z = np.zeros((64, 64), np.float32)

    def blk(a, b):
        return np.block([[a, Z], [Z, b]]).astype(np.float32)

    WaT = Wa.T.astype(np.float32)
    WbT = Wb.T.astype(np.float32)
    W2nT = (Wa - Wc).T.astype(np.float32)
    W2T = (Wc - Wa).T.astype(np.float32)
    WdT = Wd.T.astype(np.float32)
    w = np.stack([
        blk(WaT, WaT),
        blk(W2nT, W2T),
        blk(WbT, WbT),
        np.block([[WaT, Z], [-WbT, Z]]).astype(np.float32),
        blk(WdT, WdT),
        np.eye(128, dtype=np.float32),
    ])
    return np.ascontiguousarray(w, np.float32)


def run_full(X, J, Wa, Wb, Wc, Wd, trace=False, trace_kwargs=None):
    nc = _get_nc()
    wmm = _weight_stack(Wa, Wb, Wc, Wd)
    in_maps = []
    for b in range(B):
        in_maps.append({
            "XS": np.ascontiguousarray(X[b].reshape(C, 192), np.float32),
            "JS": np.ascontiguousarray(J[b].reshape(C, 192), np.float32),
            "WMM": wmm,
        })
    res = bass_utils.run_bass_kernel_spmd(
        nc, in_maps, core_ids=list(range(B)), trace=trace,
        **(trace_kwargs or {}))
    out = np.stack([res.results[b]["OUT"] for b in range(B)])
    return out.astype(np.float32), res


def kernel(X, J, Wa, Wb, Wc, Wd):
    out, _ = run_full(X, J, Wa, Wb, Wc, Wd)
    return out



# revision 3
# speedup vs baseline: 1.0974x; 1.0974x over previous
"""Trainium2 Bass kernel for nn_Complex_Only_46308337385506 (gnn_message_passing).

Math (derived + numerically validated against the jax reference):
  The per-edge orthonormal basis R (rows nU, nV, nJ) enters the output only
  through two per-edge scalars:
      gam = Jz*t,  t = 1/|J|
      a1  = sqrt(Jx^2+Jy^2)*t * sign(Jz+eps)
  With wt = gam*Xz - a1*Xx:
      Y0 = Wa@Xx + (Wa-Wc)@(a1*wt) + Wb@(gam*Xy)
      Y1 = Wa@Xy - Wb@(a1*Xz + gam*Xx)
      Y2 = Wa@Xz + (Wc-Wa)@(gam*wt) + Wb@(a1*Xy)
  followed by the VN leaky-relu stage:
      d = Wd@Y (over channel dim), dot = <Y,d>_3, dn2 = <d,d>_3
      out = Y - 0.8*min(dot,0)/(dn2+eps) * d

Sharding: data-parallel over batch B=8 -> one batch per NeuronCore.

Perf design (cost-model ~186us baseline -> target ~130us):
  - bf16 everywhere the 2e-2 tolerance allows (validated 6.2e-3 norm-rel in
    numpy): inputs are cast to bf16 host-side AND row-permuted so each
    partition's 8 point-rows are contiguous in HBM (3072B descriptors =
    full-speed DMA); output is written bf16 (1024B descriptors) and upcast
    host-side. DMA/super drops 6552ns -> 3276ns.
  - bf16 products/transposes: PE transposes cost 1 cyc/col, PSUM->SBUF
    copies hit the DVE 2x_1p mode (401ns vs 667 for f32).
  - prod slots are pre-paired (Xx,p2 | Xz,p3 | xyb,c8 | p4,p5) so each
    K=128 stationary combines two weight blocks; Y lands directly in
    xsb-layout [128=(g,f), 3, 512] PSUM via M=64 matmuls at partition
    offset 64g (5 matmuls/group), letting ONE wide ACT copy produce xsb.
  - VN scale uses TT-divide on Pool (no reciprocal+STT chain).
  - Op placement balances DVE (bf16 2x/4x ops, customs), ACT (sqrt/sign,
    wide f32 PSUM copies), Pool (flat-rate mixed-dtype muls, divide).
"""

import numpy as np
from contextlib import ExitStack

import concourse.bass as bass
import concourse.bacc as bacc
import concourse.tile as tile
from concourse import mybir
from concourse import bass_utils

F32 = mybir.dt.float32
BF16 = mybir.dt.bfloat16
AF = mybir.ActivationFunctionType
ALU = mybir.AluOpType

EPS = 1e-6
NEG = 0.2

B, C, E = 8, 16384, 64
SUPER = 1024           # points per super-iteration
NSUP = C // SUPER      # 16
GROUP = 512            # matmul free dim (points)
NCHUNK = 8             # 128-pt chunks per super


_CUSTOM_OPS = {}


def _register_custom_dve_ops():
    """Fused DVE ops (module-level, idempotent):
      SQSUM_ANT:  out = Src0^2 + Src1^2
      ADDSQ_ANT:  out = Src0 + Src1^2
      ADDSQS_ANT: out = (Src0 + Src1^2) * s0
    """
    if _CUSTOM_OPS:
        return _CUSTOM_OPS
    import numpy as _np
    from concourse import dve_ops
    from concourse.dve_spec import Spec, Src0, Src1, lower, sq, _has_src1

    def make(name, body, ref):
        spec = Spec(body=body, reference=ref)
        opcode = dve_ops._CUSTOM_DVE_ROW_BASE + len(dve_ops.OPS)
        shas = {}
        from concourse.dve_uop import DveOpSpec
        from concourse.dve_table_gen import dve_ver_for
        for ver in ("v3", "v4"):
            try:
                s = DveOpSpec(name=name, opcode=opcode,
                              uops=lower(spec, ver=ver),
                              rd1_en=_has_src1(spec))
                shas[ver] = s.sha(ver)
            except Exception:
                pass
        op = dve_ops.DveOp(name, spec, subdim=False, uops_sha=shas)
        dve_ops.OPS.append(op)
        dve_ops.CUSTOM_DVE_SPECS[name] = spec
        dve_ops._SUB_OPCODE_FOR_NAME[name] = opcode
        assert opcode < 0x20
        return op

    _CUSTOM_OPS["SQSUM"] = make(
        "SQSUM_ANT", sq(Src0) + sq(Src1),
        lambda in0, in1, s0, s1, imm2:
            (in0.astype(_np.float32) * in0 + in1.astype(_np.float32) * in1))
    _CUSTOM_OPS["ADDSQ"] = make(
        "ADDSQ_ANT", Src0 + sq(Src1),
        lambda in0, in1, s0, s1, imm2:
            in0.astype(_np.float32) + in1.astype(_np.float32) * in1)
    from concourse.dve_spec import C0
    _CUSTOM_OPS["ADDSQS"] = make(
        "ADDSQS_ANT", (Src0 + sq(Src1)) * C0,
        lambda in0, in1, s0, s1, imm2:
            (in0.astype(_np.float32) + in1.astype(_np.float32) * in1) * s0)
    return _CUSTOM_OPS


def _pin_act_table_set(arch: str):
    """Steer the ACT table-set chooser: all funcs this kernel uses must
    first-match sqrt_and_others, so exactly one table load is emitted."""
    from concourse import hw_specs
    tables = hw_specs.get_activation_tables(arch)  # cached dict, mutate in place
    mine = {AF.Sqrt, AF.Sign, AF.Copy, AF.Identity, AF.Square}
    for name, funcs in tables.items():
        if name != "sqrt_and_others":
            funcs -= mine


def _build_nc():
    global OPS
    OPS = _register_custom_dve_ops()
    nc = bacc.Bacc("TRN2", debug=False)
    _pin_act_table_set(nc.m.arch)

    XS = nc.dram_tensor("XS", [NSUP, 128, NCHUNK * 192], BF16,
                        kind="ExternalInput").ap()
    JS = nc.dram_tensor("JS", [NSUP, 128, NCHUNK * 192], BF16,
                        kind="ExternalInput").ap()
    WMM = nc.dram_tensor("WMM", [5, 128, 128], F32, kind="ExternalInput").ap()
    OUT = nc.dram_tensor("OUT", [64, 3, C], BF16, kind="ExternalOutput").ap()

    with tile.TileContext(nc) as tc, ExitStack() as ctx:
        const = ctx.enter_context(tc.tile_pool(name="const", bufs=1))
        io = ctx.enter_context(tc.tile_pool(name="io", bufs=2))
        sa = ctx.enter_context(tc.tile_pool(name="sa", bufs=2))
        prodp = ctx.enter_context(tc.tile_pool(name="prodp", bufs=2))
        rhsp = ctx.enter_context(tc.tile_pool(name="rhsp", bufs=2))
        s3p = ctx.enter_context(tc.tile_pool(name="s3p", bufs=2))
        outp = ctx.enter_context(tc.tile_pool(name="outp", bufs=2))
        psT = ctx.enter_context(tc.tile_pool(name="psT", bufs=1, space="PSUM"))
        psY = ctx.enter_context(tc.tile_pool(name="psY", bufs=1, space="PSUM"))
        psD = ctx.enter_context(tc.tile_pool(name="psD", bufs=1, space="PSUM"))

        eps_c = const.tile([128, 1], F32, tag="eps_c")
        nc.gpsimd.memset(eps_c[:], EPS)

        # weights: f32 load once, cast to bf16 (one ACT pass)
        wsb = const.tile([128, 5, 128], F32)
        nc.sync.dma_start(wsb[:], WMM.rearrange("n p m -> p n m"))
        wbf = const.tile([128, 5, 128], BF16)
        nc.scalar.activation(wbf[:], wsb[:], AF.Copy)
        W0 = wbf[:, 0, :]      # [WaT;W2nT | WaT;W2T]  (cols 0:64 -> Y0, 64:128 -> Y2)
        W1 = wbf[:, 1, :]      # blkdiag(WbT, WbT)     (cols 0:64 -> Y0, 64:128 -> Y2)
        W2s = wbf[:, 2, 0:64]  # [WaT; -WbT]           (-> Y1)
        W3 = wbf[:, 3, :]      # blkdiag(WdT, WdT)
        IDb = wbf[:, 4, :]     # identity (bf16 transposes)

        for u in range(NSUP):
            xst = io.tile([128, NCHUNK * 192], BF16, tag="xs")
            jst = io.tile([128, NCHUNK * 192], BF16, tag="js")
            nc.sync.dma_start(xst[:], XS[u])
            nc.sync.dma_start(jst[:], JS[u])
            xv = xst[:].rearrange("p (s e c) -> p s e c", s=NCHUNK, e=E, c=3)
            jv = jst[:].rearrange("p (s e c) -> p s e c", s=NCHUNK, e=E, c=3)

            def v3(t):  # [128, 512] tile -> [128, 8, 64] view
                return t[:].rearrange("p (s e) -> p s e", s=NCHUNK, e=E)

            # ---- stage A: per-edge scalars gam, a1 ---------------------
            q = sa.tile([128, SUPER // 2], BF16, tag="q")
            n2 = sa.tile([128, SUPER // 2], BF16, tag="n2")
            nc.vector._custom_dve(OPS["SQSUM"], out=v3(q),
                                  in0=jv[:, :, :, 0], in1=jv[:, :, :, 1])
            nc.vector._custom_dve(OPS["ADDSQ"], out=v3(n2),
                                  in0=v3(q), in1=jv[:, :, :, 2])
            s_ = sa.tile([128, SUPER // 2], F32, tag="s_")
            nc.scalar.activation(s_[:], n2[:], AF.Sqrt)
            sq_ = sa.tile([128, SUPER // 2], BF16, tag="sq_")
            nc.scalar.activation(sq_[:], q[:], AF.Sqrt)
            sgn = sa.tile([128, SUPER // 2], BF16, tag="sgn")
            nc.scalar.activation(v3(sgn), jv[:, :, :, 2], AF.Sign, bias=eps_c[:])
            t_ = sa.tile([128, SUPER // 2], F32, tag="t_")
            nc.vector.reciprocal_approx_fast(t_[:], s_[:])
            gam = sa.tile([128, SUPER // 2], BF16, tag="gam")
            nc.gpsimd.tensor_tensor(v3(gam), jv[:, :, :, 2], v3(t_), ALU.mult)
            a1u = sa.tile([128, SUPER // 2], BF16, tag="a1u")
            nc.gpsimd.tensor_tensor(a1u[:], sq_[:], t_[:], ALU.mult)
            a1 = sa.tile([128, SUPER // 2], BF16, tag="a1")
            nc.vector.tensor_tensor(a1[:], a1u[:], sgn[:], ALU.mult)

            # ---- stage B: products into paired prod slots --------------
            # slots: 0=Xx 1=p2 2=Xz 3=p3 4=xyb 5=c8 6=p4 7=p5
            prod = prodp.tile([128, NCHUNK, 8, E], BF16, tag="prod")
            xw = xst[:].rearrange("p (s e c) -> p s c e", s=NCHUNK, e=E, c=3)
            nc.vector.tensor_copy(prod[:, :, 0:3:2, :], xw[:, :, 0:3:2, :])
            nc.vector.tensor_copy(prod[:, :, 4, :], xv[:, :, :, 1])
            m1 = sa.tile([128, SUPER // 2], BF16, tag="m1")
            m2 = sa.tile([128, SUPER // 2], BF16, tag="m2")
            m3 = sa.tile([128, SUPER // 2], BF16, tag="m3")
            m4 = sa.tile([128, SUPER // 2], BF16, tag="m4")
            nc.gpsimd.tensor_tensor(v3(m1), v3(gam), prod[:, :, 2, :], ALU.mult)
            nc.gpsimd.tensor_tensor(v3(m2), v3(a1), prod[:, :, 0, :], ALU.mult)
            nc.gpsimd.tensor_tensor(v3(m3), v3(a1), prod[:, :, 2, :], ALU.mult)
            nc.gpsimd.tensor_tensor(v3(m4), v3(gam), prod[:, :, 0, :], ALU.mult)
            wt = sa.tile([128, SUPER // 2], BF16, tag="wt")
            nc.vector.tensor_tensor(wt[:], m1[:], m2[:], ALU.subtract)
            nc.vector.tensor_tensor(prod[:, :, 5, :], v3(m3), v3(m4), ALU.add)
            nc.vector.tensor_tensor(prod[:, :, 1, :], v3(a1), v3(wt), ALU.mult)
            nc.vector.tensor_tensor(prod[:, :, 3, :], v3(gam), v3(wt), ALU.mult)
            nc.gpsimd.tensor_tensor(prod[:, :, 6, :], v3(gam), prod[:, :, 4, :],
                                    ALU.mult)
            nc.gpsimd.tensor_tensor(prod[:, :, 7, :], v3(a1), prod[:, :, 4, :],
                                    ALU.mult)

            # ---- per group: transpose, rh copies, Y matmuls ------------
            pY = psY.tile([128, 3, GROUP], F32, tag="pY")
            for g in range(2):
                tpA = psT.tile([128, 2, GROUP], BF16, tag="tpA")
                tpB = psT.tile([128, 2, GROUP], BF16, tag="tpB")
                for k in range(4):
                    s = 4 * g + k
                    sl = slice(128 * k, 128 * (k + 1))
                    nc.tensor.transpose(tpA[:, 0, sl], prod[:, s, 0:2, :], IDb)
                    nc.tensor.transpose(tpA[:, 1, sl], prod[:, s, 2:4, :], IDb)
                    nc.tensor.transpose(tpB[:, 0, sl], prod[:, s, 4:6, :], IDb)
                    nc.tensor.transpose(tpB[:, 1, sl], prod[:, s, 6:8, :], IDb)
                rhA = rhsp.tile([128, 2, GROUP], BF16, tag="rhA")
                rhB = rhsp.tile([128, 2, GROUP], BF16, tag="rhB")
                if g == 0:
                    nc.vector.tensor_copy(rhA[:], tpA[:])
                    nc.vector.tensor_copy(rhB[:], tpB[:])
                else:
                    nc.scalar.activation(rhA[:], tpA[:], AF.Copy)
                    nc.scalar.activation(rhB[:], tpB[:], AF.Copy)

                ro = slice(64 * g, 64 * (g + 1))
                rh1 = rhA[:, 0, :]   # [Xx; p2]
                rh2 = rhA[:, 1, :]   # [Xz; p3]
                rh3 = rhB[:, 0, :]   # [xyb; c8]
                rh4 = rhB[:, 1, :]   # [p4; p5]
                nc.tensor.matmul(pY[ro, 0, :], W0[:, 0:64], rh1,
                                 start=True, stop=False)
                nc.tensor.matmul(pY[ro, 0, :], W1[:, 0:64], rh4,
                                 start=False, stop=True)
                nc.tensor.matmul(pY[ro, 2, :], W0[:, 64:128], rh2,
                                 start=True, stop=False)
                nc.tensor.matmul(pY[ro, 2, :], W1[:, 64:128], rh4,
                                 start=False, stop=True)
                nc.tensor.matmul(pY[ro, 1, :], W2s, rh3, start=True, stop=True)

            # ---- Wd stage ----------------------------------------------
            xsb = s3p.tile([128, 3, GROUP], BF16, tag="xsb")
            nc.scalar.activation(xsb[:], pY[:], AF.Copy)
            pd = psD.tile([128, 3, GROUP], F32, tag="pd")
            for i in range(3):
                nc.tensor.matmul(pd[:, i, :], W3, xsb[:, i, :], start=True,
                                 stop=True)
            dsb = s3p.tile([128, 3, GROUP], BF16, tag="dsb")
            nc.scalar.activation(dsb[:], pd[:], AF.Copy)

            # ---- VN leaky relu -----------------------------------------
            P = s3p.tile([128, 3, GROUP], BF16, tag="P")
            nc.vector.tensor_tensor(P[:], xsb[:], dsb[:], ALU.mult)
            dot = s3p.tile([128, GROUP], BF16, tag="dot")
            nc.vector.tensor_tensor(dot[:], P[:, 0, :], P[:, 1, :], ALU.add)
            dot2 = s3p.tile([128, GROUP], BF16, tag="dot2")
            nc.vector.tensor_tensor(dot2[:], dot[:], P[:, 2, :], ALU.add)
            dotm = s3p.tile([128, GROUP], BF16, tag="dotm")
            nc.vector.tensor_scalar(dotm[:], dot2[:], 0.0, None, ALU.min)

            dn2 = s3p.tile([128, GROUP], BF16, tag="dn2")
            nc.vector._custom_dve(OPS["SQSUM"], out=dn2[:],
                                  in0=dsb[:, 0, :], in1=dsb[:, 1, :])
            dn2e = s3p.tile([128, GROUP], BF16, tag="dn2e")
            nc.vector._custom_dve(OPS["ADDSQS"], out=dn2e[:],
                                  in0=dn2[:], in1=dsb[:, 2, :],
                                  s0=1.0 / (1.0 - NEG))
            dn2f = s3p.tile([128, GROUP], BF16, tag="dn2f")
            nc.vector.tensor_scalar(dn2f[:], dn2e[:], EPS / (1.0 - NEG), None,
                                    ALU.add)
            s2 = s3p.tile([128, GROUP], BF16, tag="s2")
            nc.gpsimd.tensor_tensor(s2[:], dotm[:], dn2f[:], ALU.divide)

            mi3 = s3p.tile([128, 3, GROUP], BF16, tag="mi3")
            for i in range(3):
                nc.gpsimd.tensor_tensor(mi3[:, i, :], s2[:], dsb[:, i, :],
                                        ALU.mult)
            ot = outp.tile([128, 3, GROUP], BF16, tag="ot")
            nc.vector.tensor_tensor(ot[:], xsb[:], mi3[:], ALU.subtract)

            c0 = u * SUPER
            nc.sync.dma_start(OUT[:, :, c0:c0 + GROUP], ot[0:64])
            nc.sync.dma_start(OUT[:, :, c0 + GROUP:c0 + SUPER], ot[64:128])

    nc.compile()
    return nc


_NC = None


def _get_nc():
    global _NC
    if _NC is None:
        _NC = _build_nc()
    return _NC


def _weight_stack(Wa, Wb, Wc, Wd):
    Z = np.zeros((64, 64), np.float32)
    WaT = Wa.T.astype(np.float32)
    WbT = Wb.T.astype(np.float32)
    W2nT = (Wa - Wc).T.astype(np.float32)
    W2T = (Wc - Wa).T.astype(np.float32)
    WdT = Wd.T.astype(np.float32)

    def vs(a, b):
        return np.vstack([a, b]).astype(np.float32)   # [128, 64]

    s0 = np.hstack([vs(WaT, W2nT), vs(WaT, W2T)])     # [128,128]
    s1 = np.block([[WbT, Z], [Z, WbT]]).astype(np.float32)
    s2 = np.hstack([vs(WaT, -WbT), np.zeros((128, 64), np.float32)])
    s3 = np.block([[WdT, Z], [Z, WdT]]).astype(np.float32)
    s4 = np.eye(128, dtype=np.float32)
    return np.ascontiguousarray(np.stack([s0, s1, s2, s3, s4]), np.float32)


def _prep_input(A):
    """[C, E, 3] f32 -> [NSUP, 128, NCHUNK*192] bf16 with each partition's
    8 point-rows contiguous in HBM (3072B DMA descriptors)."""
    import ml_dtypes
    Ap = A.reshape(NSUP, NCHUNK, 128, E * 3).transpose(0, 2, 1, 3)
    Ap = np.ascontiguousarray(Ap.reshape(NSUP, 128, NCHUNK * 192))
    return Ap.astype(ml_dtypes.bfloat16)


def run_full(X, J, Wa, Wb, Wc, Wd, trace=False, trace_kwargs=None):
    nc = _get_nc()
    wmm = _weight_stack(Wa, Wb, Wc, Wd)
    in_maps = []
    for b in range(B):
        in_maps.append({
            "XS": _prep_input(np.asarray(X[b], np.float32)),
            "JS": _prep_input(np.asarray(J[b], np.float32)),
            "WMM": wmm,
        })
    res = bass_utils.run_bass_kernel_spmd(
        nc, in_maps, core_ids=list(range(B)), trace=trace,
        **(trace_kwargs or {}))
    out = np.stack([np.asarray(res.results[b]["OUT"]).astype(np.float32)
                    for b in range(B)])
    return out, res


def kernel(X, J, Wa, Wb, Wc, Wd):
    out, _ = run_full(X, J, Wa, Wb, Wc, Wd)
    return out


# revision 8
# speedup vs baseline: 1.3355x; 1.2170x over previous
"""Trainium2 Bass kernel for nn_Complex_Only_46308337385506 (gnn_message_passing).

Math (derived + numerically validated against the jax reference):
  The per-edge orthonormal basis R (rows nU, nV, nJ) enters the output only
  through two per-edge scalars:
      gam = Jz*t,  t = 1/|J|
      a1  = sqrt(Jx^2+Jy^2)*t * sign(Jz+eps)
  With wt = gam*Xz - a1*Xx:
      Y0 = Wa@Xx + (Wa-Wc)@(a1*wt) + Wb@(gam*Xy)
      Y1 = Wa@Xy - Wb@(a1*Xz + gam*Xx)
      Y2 = Wa@Xz + (Wc-Wa)@(gam*wt) + Wb@(a1*Xy)
  followed by the VN leaky-relu stage:
      d = Wd@Y (over channel dim), dot = <Y,d>_3, dn2 = <d,d>_3
      out = Y - 0.8*min(dot,0)/(dn2+eps) * d

Sharding: data-parallel over batch B=8 -> one batch per NeuronCore.

Perf design (cost-model ~186us baseline -> target ~130us):
  - bf16 everywhere the 2e-2 tolerance allows (validated 6.2e-3 norm-rel in
    numpy): inputs are cast to bf16 host-side AND row-permuted so each
    partition's 8 point-rows are contiguous in HBM (3072B descriptors =
    full-speed DMA); output is written bf16 (1024B descriptors) and upcast
    host-side. DMA/super drops 6552ns -> 3276ns.
  - bf16 products/transposes: PE transposes cost 1 cyc/col, PSUM->SBUF
    copies hit the DVE 2x_1p mode (401ns vs 667 for f32).
  - prod slots are pre-paired (Xx,p2 | Xz,p3 | xyb,c8 | p4,p5) so each
    K=128 stationary combines two weight blocks; Y lands directly in
    xsb-layout [128=(g,f), 3, 512] PSUM via M=64 matmuls at partition
    offset 64g (5 matmuls/group), letting ONE wide ACT copy produce xsb.
  - VN scale uses TT-divide on Pool (no reciprocal+STT chain).
  - Op placement balances DVE (bf16 2x/4x ops, customs), ACT (sqrt/sign,
    wide f32 PSUM copies), Pool (flat-rate mixed-dtype muls, divide).
"""

import numpy as np
from contextlib import ExitStack

import concourse.bass as bass
import concourse.bacc as bacc
import concourse.tile as tile
from concourse import mybir
from concourse import bass_utils

F32 = mybir.dt.float32
BF16 = mybir.dt.bfloat16
AF = mybir.ActivationFunctionType
ALU = mybir.AluOpType

EPS = 1e-6
NEG = 0.2

B, C, E = 8, 16384, 64
SUPER = 1024           # points per super-iteration
NSUP = C // SUPER      # 16
GROUP = 512            # matmul free dim (points)
NCHUNK = 8             # 128-pt chunks per super


_CUSTOM_OPS = {}


def _register_custom_dve_ops():
    """Fused DVE ops (module-level, idempotent):
      SQSUM_ANT:  out = Src0^2 + Src1^2
      ADDSQ_ANT:  out = Src0 + Src1^2
      ADDSQS_ANT: out = (Src0 + Src1^2) * s0
    """
    if _CUSTOM_OPS:
        return _CUSTOM_OPS
    import numpy as _np
    from concourse import dve_ops
    from concourse.dve_spec import Spec, Src0, Src1, lower, sq, _has_src1

    def make(name, body, ref):
        spec = Spec(body=body, reference=ref)
        opcode = dve_ops._CUSTOM_DVE_ROW_BASE + len(dve_ops.OPS)
        shas = {}
        from concourse.dve_uop import DveOpSpec
        from concourse.dve_table_gen import dve_ver_for
        for ver in ("v3", "v4"):
            try:
                s = DveOpSpec(name=name, opcode=opcode,
                              uops=lower(spec, ver=ver),
                              rd1_en=_has_src1(spec))
                shas[ver] = s.sha(ver)
            except Exception:
                pass
        op = dve_ops.DveOp(name, spec, subdim=False, uops_sha=shas)
        dve_ops.OPS.append(op)
        dve_ops.CUSTOM_DVE_SPECS[name] = spec
        dve_ops._SUB_OPCODE_FOR_NAME[name] = opcode
        assert opcode < 0x20
        return op

    _CUSTOM_OPS["SQSUM"] = make(
        "SQSUM_ANT", sq(Src0) + sq(Src1),
        lambda in0, in1, s0, s1, imm2:
            (in0.astype(_np.float32) * in0 + in1.astype(_np.float32) * in1))
    _CUSTOM_OPS["ADDSQ"] = make(
        "ADDSQ_ANT", Src0 + sq(Src1),
        lambda in0, in1, s0, s1, imm2:
            in0.astype(_np.float32) + in1.astype(_np.float32) * in1)
    from concourse.dve_spec import C0
    _CUSTOM_OPS["ADDSQS"] = make(
        "ADDSQS_ANT", (Src0 + sq(Src1)) * C0,
        lambda in0, in1, s0, s1, imm2:
            (in0.astype(_np.float32) + in1.astype(_np.float32) * in1) * s0)
    return _CUSTOM_OPS


def _pin_act_table_set(arch: str):
    """Steer the ACT table-set chooser: all funcs this kernel uses must
    first-match sqrt_and_others, so exactly one table load is emitted."""
    from concourse import hw_specs
    tables = hw_specs.get_activation_tables(arch)  # cached dict, mutate in place
    mine = {AF.Sqrt, AF.Sign, AF.Copy, AF.Identity, AF.Square}
    for name, funcs in tables.items():
        if name != "sqrt_and_others":
            funcs -= mine


def _build_nc():
    global OPS
    OPS = _register_custom_dve_ops()
    nc = bacc.Bacc("TRN2", debug=False)
    _pin_act_table_set(nc.m.arch)

    XS = nc.dram_tensor("XS", [NSUP, 128, NCHUNK * 192], BF16,
                        kind="ExternalInput").ap()
    JS = nc.dram_tensor("JS", [NSUP, 128, NCHUNK * 192], BF16,
                        kind="ExternalInput").ap()
    WMM = nc.dram_tensor("WMM", [5, 128, 128], F32, kind="ExternalInput").ap()
    OUT = nc.dram_tensor("OUT", [64, 3, C], BF16, kind="ExternalOutput").ap()

    with tile.TileContext(nc) as tc, ExitStack() as ctx:
        const = ctx.enter_context(tc.tile_pool(name="const", bufs=1))
        io = ctx.enter_context(tc.tile_pool(name="io", bufs=2))
        sa = ctx.enter_context(tc.tile_pool(name="sa", bufs=2))
        prodp = ctx.enter_context(tc.tile_pool(name="prodp", bufs=2))
        rhsp = ctx.enter_context(tc.tile_pool(name="rhsp", bufs=2))
        s3p = ctx.enter_context(tc.tile_pool(name="s3p", bufs=2))
        outp = ctx.enter_context(tc.tile_pool(name="outp", bufs=2))
        psT = ctx.enter_context(tc.tile_pool(name="psT", bufs=1, space="PSUM"))
        psY = ctx.enter_context(tc.tile_pool(name="psY", bufs=1, space="PSUM"))
        psD = ctx.enter_context(tc.tile_pool(name="psD", bufs=1, space="PSUM"))

        eps_c = const.tile([128, 1], F32, tag="eps_c")
        nc.gpsimd.memset(eps_c[:], EPS)

        # weights: f32 load once, cast to bf16 (one ACT pass)
        wsb = const.tile([128, 5, 128], F32)
        nc.sync.dma_start(wsb[:], WMM.rearrange("n p m -> p n m"))
        wbf = const.tile([128, 5, 128], BF16)
        nc.scalar.activation(wbf[:], wsb[:], AF.Copy)
        W0 = wbf[:, 0, :]      # [WaT;W2nT | WaT;W2T]  (cols 0:64 -> Y0, 64:128 -> Y2)
        W1 = wbf[:, 1, :]      # blkdiag(WbT, WbT)     (cols 0:64 -> Y0, 64:128 -> Y2)
        W2s = wbf[:, 2, 0:64]  # [WaT; -WbT]           (-> Y1)
        W3 = wbf[:, 3, :]      # blkdiag(WdT, WdT)
        IDb = wbf[:, 4, :]     # identity (bf16 transposes)

        for u in range(NSUP):
            xst = io.tile([128, NCHUNK * 192], BF16, tag="xs")
            jst = io.tile([128, NCHUNK * 192], BF16, tag="js")
            nc.sync.dma_start(xst[:], XS[u])
            nc.sync.dma_start(jst[:], JS[u])
            xv = xst[:].rearrange("p (s e c) -> p s e c", s=NCHUNK, e=E, c=3)
            jv = jst[:].rearrange("p (s e c) -> p s e c", s=NCHUNK, e=E, c=3)

            def v3(t):  # [128, 512] tile -> [128, 8, 64] view
                return t[:].rearrange("p (s e) -> p s e", s=NCHUNK, e=E)

            # ---- stage A: per-edge scalars gam, a1 ---------------------
            q = sa.tile([128, SUPER // 2], BF16, tag="q")
            n2 = sa.tile([128, SUPER // 2], BF16, tag="n2")
            nc.vector._custom_dve(OPS["SQSUM"], out=v3(q),
                                  in0=jv[:, :, :, 0], in1=jv[:, :, :, 1])
            nc.vector._custom_dve(OPS["ADDSQ"], out=v3(n2),
                                  in0=v3(q), in1=jv[:, :, :, 2])
            s_ = sa.tile([128, SUPER // 2], F32, tag="s_")
            nc.scalar.activation(s_[:], n2[:], AF.Sqrt)
            sq_ = sa.tile([128, SUPER // 2], BF16, tag="sq_")
            nc.scalar.activation(sq_[:], q[:], AF.Sqrt)
            sgn = sa.tile([128, SUPER // 2], BF16, tag="sgn")
            nc.scalar.activation(v3(sgn), jv[:, :, :, 2], AF.Sign, bias=eps_c[:])
            gam = sa.tile([128, SUPER // 2], BF16, tag="gam")
            nc.gpsimd.tensor_tensor(v3(gam), jv[:, :, :, 2], v3(s_), ALU.divide)
            a1u = sa.tile([128, SUPER // 2], BF16, tag="a1u")
            nc.gpsimd.tensor_tensor(a1u[:], sq_[:], s_[:], ALU.divide)
            a1 = sa.tile([128, SUPER // 2], BF16, tag="a1")
            nc.vector.tensor_tensor(a1[:], a1u[:], sgn[:], ALU.mult)

            # ---- stage B: products into paired prod slots --------------
            # slots: 0=Xx 1=p2 2=xyb 3=c8 4=Xz 5=p3 6=p4 7=p5
            prod = prodp.tile([128, NCHUNK, 8, E], BF16, tag="prod")
            xw = xst[:].rearrange("p (s e c) -> p s c e", s=NCHUNK, e=E, c=3)
            nc.vector.tensor_copy(prod[:, :, 0:5:2, :], xw[:])
            m1 = sa.tile([128, SUPER // 2], BF16, tag="m1")
            m2 = sa.tile([128, SUPER // 2], BF16, tag="m2")
            m3 = sa.tile([128, SUPER // 2], BF16, tag="m3")
            m4 = sa.tile([128, SUPER // 2], BF16, tag="m4")
            nc.gpsimd.tensor_tensor(v3(m1), v3(gam), prod[:, :, 4, :], ALU.mult)
            nc.gpsimd.tensor_tensor(v3(m2), v3(a1), prod[:, :, 0, :], ALU.mult)
            nc.gpsimd.tensor_tensor(v3(m3), v3(a1), prod[:, :, 4, :], ALU.mult)
            nc.gpsimd.tensor_tensor(v3(m4), v3(gam), prod[:, :, 0, :], ALU.mult)
            wt = sa.tile([128, SUPER // 2], BF16, tag="wt")
            nc.vector.tensor_tensor(wt[:], m1[:], m2[:], ALU.subtract)
            nc.vector.tensor_tensor(prod[:, :, 3, :], v3(m3), v3(m4), ALU.add)
            nc.vector.tensor_tensor(prod[:, :, 1, :], v3(a1), v3(wt), ALU.mult)
            nc.vector.tensor_tensor(prod[:, :, 5, :], v3(gam), v3(wt), ALU.mult)
            nc.gpsimd.tensor_tensor(prod[:, :, 6, :], v3(gam), prod[:, :, 2, :],
                                    ALU.mult)
            nc.gpsimd.tensor_tensor(prod[:, :, 7, :], v3(a1), prod[:, :, 2, :],
                                    ALU.mult)

            # ---- per group: transpose, rh copies, Y matmuls ------------
            pY = psY.tile([128, 3, GROUP], F32, tag="pY")
            for g in range(2):
                tpA = psT.tile([128, 2, GROUP], BF16, tag="tpA")
                tpB = psT.tile([128, 2, GROUP], BF16, tag="tpB")
                for k in range(4):
                    s = 4 * g + k
                    sl = slice(128 * k, 128 * (k + 1))
                    nc.tensor.transpose(tpA[:, 0, sl], prod[:, s, 0:2, :], IDb)
                    nc.tensor.transpose(tpA[:, 1, sl], prod[:, s, 4:6, :], IDb)
                    nc.tensor.transpose(tpB[:, 0, sl], prod[:, s, 2:4, :], IDb)
                    nc.tensor.transpose(tpB[:, 1, sl], prod[:, s, 6:8, :], IDb)
                rhA = rhsp.tile([128, 2, GROUP], BF16, tag="rhA")
                rhB = rhsp.tile([128, 2, GROUP], BF16, tag="rhB")
                if g == 0:
                    nc.vector.tensor_copy(rhA[:], tpA[:])
                    nc.vector.tensor_copy(rhB[:], tpB[:])
                else:
                    nc.scalar.activation(rhA[:], tpA[:], AF.Copy)
                    nc.scalar.activation(rhB[:], tpB[:], AF.Copy)

                ro = slice(64 * g, 64 * (g + 1))
                rh1 = rhA[:, 0, :]   # [Xx; p2]
                rh2 = rhA[:, 1, :]   # [Xz; p3]
                rh3 = rhB[:, 0, :]   # [xyb; c8]
                rh4 = rhB[:, 1, :]   # [p4; p5]
                nc.tensor.matmul(pY[ro, 0, :], W0[:, 0:64], rh1,
                                 start=True, stop=False)
                nc.tensor.matmul(pY[ro, 0, :], W1[:, 0:64], rh4,
                                 start=False, stop=True)
                nc.tensor.matmul(pY[ro, 2, :], W0[:, 64:128], rh2,
                                 start=True, stop=False)
                nc.tensor.matmul(pY[ro, 2, :], W1[:, 64:128], rh4,
                                 start=False, stop=True)
                nc.tensor.matmul(pY[ro, 1, :], W2s, rh3, start=True, stop=True)

            # ---- Wd stage ----------------------------------------------
            xsb = s3p.tile([128, 3, GROUP], BF16, tag="xsb")
            nc.scalar.activation(xsb[:], pY[:], AF.Copy)
            pd = psD.tile([128, 3, GROUP], F32, tag="pd")
            for i in range(3):
                nc.tensor.matmul(pd[:, i, :], W3, xsb[:, i, :], start=True,
                                 stop=True)
            dsb = s3p.tile([128, 3, GROUP], BF16, tag="dsb")
            nc.scalar.activation(dsb[:], pd[:], AF.Copy)

            # ---- VN leaky relu -----------------------------------------
            P = s3p.tile([128, 3, GROUP], BF16, tag="P")
            nc.vector.tensor_tensor(P[:], xsb[:], dsb[:], ALU.mult)
            dot = s3p.tile([128, GROUP], BF16, tag="dot")
            nc.gpsimd.tensor_tensor(dot[:], P[:, 0, :], P[:, 1, :], ALU.add)
            dot2 = s3p.tile([128, GROUP], BF16, tag="dot2")
            nc.gpsimd.tensor_tensor(dot2[:], dot[:], P[:, 2, :], ALU.add)
            dotm = s3p.tile([128, GROUP], BF16, tag="dotm")
            nc.vector.tensor_scalar(dotm[:], dot2[:], 0.0, None, ALU.min)

            dn2 = s3p.tile([128, GROUP], BF16, tag="dn2")
            nc.vector._custom_dve(OPS["SQSUM"], out=dn2[:],
                                  in0=dsb[:, 0, :], in1=dsb[:, 1, :])
            dn2e = s3p.tile([128, GROUP], BF16, tag="dn2e")
            nc.vector._custom_dve(OPS["ADDSQS"], out=dn2e[:],
                                  in0=dn2[:], in1=dsb[:, 2, :],
                                  s0=1.0 / (1.0 - NEG))
            dn2f = s3p.tile([128, GROUP], BF16, tag="dn2f")
            nc.vector.tensor_scalar(dn2f[:], dn2e[:], EPS / (1.0 - NEG), None,
                                    ALU.add)
            s2 = s3p.tile([128, GROUP], BF16, tag="s2")
            nc.gpsimd.tensor_tensor(s2[:], dotm[:], dn2f[:], ALU.divide)

            mi3 = s3p.tile([128, 3, GROUP], BF16, tag="mi3")
            for i in range(3):
                nc.gpsimd.tensor_tensor(mi3[:, i, :], s2[:], dsb[:, i, :],
                                        ALU.mult)
            ot = outp.tile([128, 3, GROUP], BF16, tag="ot")
            nc.gpsimd.tensor_tensor(ot[:], xsb[:], mi3[:], ALU.subtract)

            c0 = u * SUPER
            nc.sync.dma_start(OUT[:, :, c0:c0 + GROUP], ot[0:64])
            nc.sync.dma_start(OUT[:, :, c0 + GROUP:c0 + SUPER], ot[64:128])

    nc.compile()
    return nc


_NC = None


def _get_nc():
    global _NC
    if _NC is None:
        _NC = _build_nc()
    return _NC


def _weight_stack(Wa, Wb, Wc, Wd):
    Z = np.zeros((64, 64), np.float32)
    WaT = Wa.T.astype(np.float32)
    WbT = Wb.T.astype(np.float32)
    W2nT = (Wa - Wc).T.astype(np.float32)
    W2T = (Wc - Wa).T.astype(np.float32)
    WdT = Wd.T.astype(np.float32)

    def vs(a, b):
        return np.vstack([a, b]).astype(np.float32)   # [128, 64]

    s0 = np.hstack([vs(WaT, W2nT), vs(WaT, W2T)])     # [128,128]
    s1 = np.block([[WbT, Z], [Z, WbT]]).astype(np.float32)
    s2 = np.hstack([vs(WaT, -WbT), np.zeros((128, 64), np.float32)])
    s3 = np.block([[WdT, Z], [Z, WdT]]).astype(np.float32)
    s4 = np.eye(128, dtype=np.float32)
    return np.ascontiguousarray(np.stack([s0, s1, s2, s3, s4]), np.float32)


def _prep_input(A):
    """[C, E, 3] f32 -> [NSUP, 128, NCHUNK*192] bf16 with each partition's
    8 point-rows contiguous in HBM (3072B DMA descriptors)."""
    import ml_dtypes
    Ap = A.reshape(NSUP, NCHUNK, 128, E * 3).transpose(0, 2, 1, 3)
    Ap = np.ascontiguousarray(Ap.reshape(NSUP, 128, NCHUNK * 192))
    return Ap.astype(ml_dtypes.bfloat16)


def run_full(X, J, Wa, Wb, Wc, Wd, trace=False, trace_kwargs=None):
    nc = _get_nc()
    wmm = _weight_stack(Wa, Wb, Wc, Wd)
    in_maps = []
    for b in range(B):
        in_maps.append({
            "XS": _prep_input(np.asarray(X[b], np.float32)),
            "JS": _prep_input(np.asarray(J[b], np.float32)),
            "WMM": wmm,
        })
    res = bass_utils.run_bass_kernel_spmd(
        nc, in_maps, core_ids=list(range(B)), trace=trace,
        **(trace_kwargs or {}))
    out = np.stack([np.asarray(res.results[b]["OUT"]).astype(np.float32)
                    for b in range(B)])
    return out, res


def kernel(X, J, Wa, Wb, Wc, Wd):
    out, _ = run_full(X, J, Wa, Wb, Wc, Wd)
    return out


# revision 15
# speedup vs baseline: 1.3841x; 1.0364x over previous
"""Trainium2 Bass kernel for nn_Complex_Only_46308337385506 (gnn_message_passing).

Math (derived + numerically validated against the jax reference):
  The per-edge orthonormal basis R (rows nU, nV, nJ) enters the output only
  through two per-edge scalars:
      gam = Jz*t,  t = 1/|J|
      a1  = sqrt(Jx^2+Jy^2)*t * sign(Jz+eps)
  With wt = gam*Xz - a1*Xx:
      Y0 = Wa@Xx + (Wa-Wc)@(a1*wt) + Wb@(gam*Xy)
      Y1 = Wa@Xy - Wb@(a1*Xz + gam*Xx)
      Y2 = Wa@Xz + (Wc-Wa)@(gam*wt) + Wb@(a1*Xy)
  followed by the VN leaky-relu stage:
      d = Wd@Y (over channel dim), dot = <Y,d>_3, dn2 = <d,d>_3
      out = Y - 0.8*min(dot,0)/(dn2+eps) * d

Sharding: data-parallel over batch B=8 -> one batch per NeuronCore.

Perf design (cost-model ~186us baseline -> target ~130us):
  - bf16 everywhere the 2e-2 tolerance allows (validated 6.2e-3 norm-rel in
    numpy): inputs are cast to bf16 host-side AND row-permuted so each
    partition's 8 point-rows are contiguous in HBM (3072B descriptors =
    full-speed DMA); output is written bf16 (1024B descriptors) and upcast
    host-side. DMA/super drops 6552ns -> 3276ns.
  - bf16 products/transposes: PE transposes cost 1 cyc/col, PSUM->SBUF
    copies hit the DVE 2x_1p mode (401ns vs 667 for f32).
  - prod slots are pre-paired (Xx,p2 | Xz,p3 | xyb,c8 | p4,p5) so each
    K=128 stationary combines two weight blocks; Y lands directly in
    xsb-layout [128=(g,f), 3, 512] PSUM via M=64 matmuls at partition
    offset 64g (5 matmuls/group), letting ONE wide ACT copy produce xsb.
  - VN scale uses TT-divide on Pool (no reciprocal+STT chain).
  - Op placement balances DVE (bf16 2x/4x ops, customs), ACT (sqrt/sign,
    wide f32 PSUM copies), Pool (flat-rate mixed-dtype muls, divide).
"""

import numpy as np
from contextlib import ExitStack

import concourse.bass as bass
import concourse.bacc as bacc
import concourse.tile as tile
from concourse import mybir
from concourse import bass_utils

F32 = mybir.dt.float32
BF16 = mybir.dt.bfloat16
AF = mybir.ActivationFunctionType
ALU = mybir.AluOpType

EPS = 1e-6
NEG = 0.2

B, C, E = 8, 16384, 64
SUPER = 1024           # points per super-iteration
NSUP = C // SUPER      # 16
GROUP = 512            # matmul free dim (points)
NCHUNK = 8             # 128-pt chunks per super


_CUSTOM_OPS = {}


def _register_custom_dve_ops():
    """Fused DVE ops (module-level, idempotent):
      SQSUM_ANT:  out = Src0^2 + Src1^2
      ADDSQ_ANT:  out = Src0 + Src1^2
      ADDSQS_ANT: out = (Src0 + Src1^2) * s0
    """
    if _CUSTOM_OPS:
        return _CUSTOM_OPS
    import numpy as _np
    from concourse import dve_ops
    from concourse.dve_spec import Spec, Src0, Src1, lower, sq, _has_src1

    def make(name, body, ref):
        spec = Spec(body=body, reference=ref)
        opcode = dve_ops._CUSTOM_DVE_ROW_BASE + len(dve_ops.OPS)
        shas = {}
        from concourse.dve_uop import DveOpSpec
        from concourse.dve_table_gen import dve_ver_for
        for ver in ("v3", "v4"):
            try:
                s = DveOpSpec(name=name, opcode=opcode,
                              uops=lower(spec, ver=ver),
                              rd1_en=_has_src1(spec))
                shas[ver] = s.sha(ver)
            except Exception:
                pass
        op = dve_ops.DveOp(name, spec, subdim=False, uops_sha=shas)
        dve_ops.OPS.append(op)
        dve_ops.CUSTOM_DVE_SPECS[name] = spec
        dve_ops._SUB_OPCODE_FOR_NAME[name] = opcode
        assert opcode < 0x20
        return op

    def fl(a):
        # operand APs may lower with different (collapsed vs not) free-dim
        # shapes; flatten to [p, -1] (same row-major order) before combining
        return _np.asarray(a).reshape(_np.asarray(a).shape[0], -1)

    _CUSTOM_OPS["SQSUM"] = make(
        "SQSUM_ANT", sq(Src0) + sq(Src1),
        lambda in0, in1, s0, s1, imm2:
            (fl(in0).astype(_np.float32) ** 2 + fl(in1).astype(_np.float32) ** 2))
    _CUSTOM_OPS["ADDSQ"] = make(
        "ADDSQ_ANT", Src0 + sq(Src1),
        lambda in0, in1, s0, s1, imm2:
            fl(in0).astype(_np.float32) + fl(in1).astype(_np.float32) ** 2)
    from concourse.dve_spec import C0
    _CUSTOM_OPS["ADDSQS"] = make(
        "ADDSQS_ANT", (Src0 + sq(Src1)) * C0,
        lambda in0, in1, s0, s1, imm2:
            (fl(in0).astype(_np.float32) + fl(in1).astype(_np.float32) ** 2) * s0)
    return _CUSTOM_OPS


def _pin_act_table_set(arch: str):
    """Steer the ACT table-set chooser: all funcs this kernel uses must
    first-match sqrt_and_others, so exactly one table load is emitted."""
    from concourse import hw_specs
    tables = hw_specs.get_activation_tables(arch)  # cached dict, mutate in place
    mine = {AF.Sqrt, AF.Sign, AF.Copy, AF.Identity, AF.Square}
    for name, funcs in tables.items():
        if name != "sqrt_and_others":
            funcs -= mine


def _build_nc():
    global OPS
    OPS = _register_custom_dve_ops()
    nc = bacc.Bacc("TRN2", debug=False)
    _pin_act_table_set(nc.m.arch)

    XS = nc.dram_tensor("XS", [NSUP, 128, NCHUNK * 192], BF16,
                        kind="ExternalInput").ap()
    JS = nc.dram_tensor("JS", [NSUP, 128, NCHUNK * 192], BF16,
                        kind="ExternalInput").ap()
    WMM = nc.dram_tensor("WMM", [5, 128, 128], F32, kind="ExternalInput").ap()
    OUT = nc.dram_tensor("OUT", [64, 3, C], BF16, kind="ExternalOutput").ap()

    with tile.TileContext(nc) as tc, ExitStack() as ctx:
        const = ctx.enter_context(tc.tile_pool(name="const", bufs=1))
        io = ctx.enter_context(tc.tile_pool(name="io", bufs=2))
        sa = ctx.enter_context(tc.tile_pool(name="sa", bufs=2))
        prodp = ctx.enter_context(tc.tile_pool(name="prodp", bufs=2))
        rhsp = ctx.enter_context(tc.tile_pool(name="rhsp", bufs=2))
        s3p = ctx.enter_context(tc.tile_pool(name="s3p", bufs=2))
        outp = ctx.enter_context(tc.tile_pool(name="outp", bufs=2))
        psT = ctx.enter_context(tc.tile_pool(name="psT", bufs=1, space="PSUM"))
        psY = ctx.enter_context(tc.tile_pool(name="psY", bufs=1, space="PSUM"))
        psD = ctx.enter_context(tc.tile_pool(name="psD", bufs=1, space="PSUM"))

        eps_c = const.tile([128, 1], F32, tag="eps_c")
        nc.gpsimd.memset(eps_c[:], EPS)

        # weights: f32 load once, cast to bf16 (one ACT pass)
        wsb = const.tile([128, 5, 128], F32)
        nc.sync.dma_start(wsb[:], WMM.rearrange("n p m -> p n m"))
        wbf = const.tile([128, 5, 128], BF16)
        nc.scalar.activation(wbf[:], wsb[:], AF.Copy)
        W0 = wbf[:, 0, :]      # [WaT;W2nT | WaT;W2T]  (cols 0:64 -> Y0, 64:128 -> Y2)
        W1 = wbf[:, 1, :]      # blkdiag(WbT, WbT)     (cols 0:64 -> Y0, 64:128 -> Y2)
        W2s = wbf[:, 2, 0:64]  # [WaT; -WbT]           (-> Y1)
        W3 = wbf[:, 3, :]      # blkdiag(WdT, WdT)
        IDb = wbf[:, 4, :]     # identity (bf16 transposes)

        for u in range(NSUP):
            xst = io.tile([128, NCHUNK * 192], BF16, tag="xs")
            jst = io.tile([128, NCHUNK * 192], BF16, tag="js")
            nc.sync.dma_start(xst[:], XS[u])
            nc.sync.dma_start(jst[:], JS[u])
            # host layout is [s, c, e]: every field slice is packed (innermost e)
            xv = xst[:].rearrange("p (s c e) -> p s c e", s=NCHUNK, c=3, e=E)
            jv = jst[:].rearrange("p (s c e) -> p s c e", s=NCHUNK, c=3, e=E)

            def v3(t):  # [128, 512] tile -> [128, 8, 64] view
                return t[:].rearrange("p (s e) -> p s e", s=NCHUNK, e=E)

            # ---- stage A: per-edge scalars gam, a1 ---------------------
            q = sa.tile([128, SUPER // 2], BF16, tag="q")
            n2 = sa.tile([128, SUPER // 2], BF16, tag="n2")
            nc.vector._custom_dve(OPS["SQSUM"], out=v3(q),
                                  in0=jv[:, :, 0, :], in1=jv[:, :, 1, :]
                                  ).ins.perf_max = 2
            nc.vector._custom_dve(OPS["ADDSQ"], out=v3(n2),
                                  in0=v3(q), in1=jv[:, :, 2, :]
                                  ).ins.perf_max = 2
            s_ = sa.tile([128, SUPER // 2], F32, tag="s_")
            nc.scalar.activation(s_[:], n2[:], AF.Sqrt)
            sq_ = sa.tile([128, SUPER // 2], BF16, tag="sq_")
            nc.scalar.activation(sq_[:], q[:], AF.Sqrt)
            sgn = sa.tile([128, SUPER // 2], BF16, tag="sgn")
            nc.scalar.activation(v3(sgn), jv[:, :, 2, :], AF.Sign, bias=eps_c[:])
            gam = sa.tile([128, SUPER // 2], BF16, tag="gam")
            nc.gpsimd.tensor_tensor(v3(gam), jv[:, :, 2, :], v3(s_), ALU.divide)
            a1u = sa.tile([128, SUPER // 2], BF16, tag="a1u")
            nc.gpsimd.tensor_tensor(a1u[:], sq_[:], s_[:], ALU.divide)
            a1 = sa.tile([128, SUPER // 2], BF16, tag="a1")
            nc.vector.tensor_tensor(a1[:], a1u[:], sgn[:], ALU.mult)

            # ---- stage B: products into paired prod slots --------------
            # slots: 0=Xx 1=p2 2=xyb 3=c8 4=Xz 5=p3 6=p4 7=p5
            prod = prodp.tile([128, NCHUNK, 8, E], BF16, tag="prod")
            nc.vector.tensor_copy(prod[:, :, 0:5:2, :], xv[:])
            m1 = sa.tile([128, SUPER // 2], BF16, tag="m1")
            m2 = sa.tile([128, SUPER // 2], BF16, tag="m2")
            m3 = sa.tile([128, SUPER // 2], BF16, tag="m3")
            m4 = sa.tile([128, SUPER // 2], BF16, tag="m4")
            nc.gpsimd.tensor_tensor(v3(m1), v3(gam), prod[:, :, 4, :], ALU.mult)
            nc.gpsimd.tensor_tensor(v3(m2), v3(a1), prod[:, :, 0, :], ALU.mult)
            nc.gpsimd.tensor_tensor(v3(m3), v3(a1), prod[:, :, 4, :], ALU.mult)
            nc.gpsimd.tensor_tensor(v3(m4), v3(gam), prod[:, :, 0, :], ALU.mult)
            wt = sa.tile([128, SUPER // 2], BF16, tag="wt")
            nc.vector.tensor_tensor(wt[:], m1[:], m2[:], ALU.subtract)
            nc.vector.tensor_tensor(prod[:, :, 3, :], v3(m3), v3(m4), ALU.add)
            nc.vector.tensor_tensor(prod[:, :, 1, :], v3(a1), v3(wt), ALU.mult)
            nc.vector.tensor_tensor(prod[:, :, 5, :], v3(gam), v3(wt), ALU.mult)
            nc.gpsimd.tensor_tensor(prod[:, :, 6, :], v3(gam), prod[:, :, 2, :],
                                    ALU.mult)
            nc.gpsimd.tensor_tensor(prod[:, :, 7, :], v3(a1), prod[:, :, 2, :],
                                    ALU.mult)

            # ---- per group: transpose, rh copies, Y matmuls ------------
            pY = psY.tile([128, 3, GROUP], F32, tag="pY")
            for g in range(2):
                tpA = psT.tile([128, 2, GROUP], BF16, tag="tpA")
                tpB = psT.tile([128, 2, GROUP], BF16, tag="tpB")
                for k in range(4):
                    s = 4 * g + k
                    sl = slice(128 * k, 128 * (k + 1))
                    nc.tensor.transpose(tpA[:, 0, sl], prod[:, s, 0:2, :], IDb)
                    nc.tensor.transpose(tpA[:, 1, sl], prod[:, s, 4:6, :], IDb)
                    nc.tensor.transpose(tpB[:, 0, sl], prod[:, s, 2:4, :], IDb)
                    nc.tensor.transpose(tpB[:, 1, sl], prod[:, s, 6:8, :], IDb)
                rhA = rhsp.tile([128, 2, GROUP], BF16, tag="rhA")
                rhB = rhsp.tile([128, 2, GROUP], BF16, tag="rhB")
                if g == 0:
                    nc.vector.tensor_copy(rhA[:], tpA[:])
                    nc.vector.tensor_copy(rhB[:], tpB[:])
                else:
                    nc.scalar.activation(rhA[:], tpA[:], AF.Copy)
                    nc.scalar.activation(rhB[:], tpB[:], AF.Copy)

                ro = slice(64 * g, 64 * (g + 1))
                rh1 = rhA[:, 0, :]   # [Xx; p2]
                rh2 = rhA[:, 1, :]   # [Xz; p3]
                rh3 = rhB[:, 0, :]   # [xyb; c8]
                rh4 = rhB[:, 1, :]   # [p4; p5]
                nc.tensor.matmul(pY[ro, 0, :], W0[:, 0:64], rh1,
                                 start=True, stop=False)
                nc.tensor.matmul(pY[ro, 0, :], W1[:, 0:64], rh4,
                                 start=False, stop=True)
                nc.tensor.matmul(pY[ro, 2, :], W0[:, 64:128], rh2,
                                 start=True, stop=False)
                nc.tensor.matmul(pY[ro, 2, :], W1[:, 64:128], rh4,
                                 start=False, stop=True)
                nc.tensor.matmul(pY[ro, 1, :], W2s, rh3, start=True, stop=True)

            # ---- Wd stage ----------------------------------------------
            xsb = s3p.tile([128, 3, GROUP], BF16, tag="xsb")
            nc.scalar.activation(xsb[:], pY[:], AF.Copy)
            pd = psD.tile([128, 3, GROUP], F32, tag="pd")
            for i in range(3):
                nc.tensor.matmul(pd[:, i, :], W3, xsb[:, i, :], start=True,
                                 stop=True)
            dsb = s3p.tile([128, 3, GROUP], BF16, tag="dsb")
            nc.scalar.activation(dsb[:], pd[:], AF.Copy)

            # ---- VN leaky relu -----------------------------------------
            P = s3p.tile([128, 3, GROUP], BF16, tag="P")
            nc.vector.tensor_tensor(P[:], xsb[:], dsb[:], ALU.mult)
            dot = s3p.tile([128, GROUP], BF16, tag="dot")
            nc.vector.tensor_tensor(dot[:], P[:, 0, :], P[:, 1, :], ALU.add)
            dot2 = s3p.tile([128, GROUP], BF16, tag="dot2")
            nc.vector.tensor_tensor(dot2[:], dot[:], P[:, 2, :], ALU.add)
            dotm = s3p.tile([128, GROUP], BF16, tag="dotm")
            nc.vector.tensor_scalar(dotm[:], dot2[:], 0.0, None, ALU.min)

            dn2 = s3p.tile([128, GROUP], BF16, tag="dn2")
            nc.vector._custom_dve(OPS["SQSUM"], out=dn2[:],
                                  in0=dsb[:, 0, :], in1=dsb[:, 1, :]
                                  ).ins.perf_max = 2
            dn2e = s3p.tile([128, GROUP], BF16, tag="dn2e")
            nc.vector._custom_dve(OPS["ADDSQS"], out=dn2e[:],
                                  in0=dn2[:], in1=dsb[:, 2, :],
                                  s0=1.0 / (1.0 - NEG)).ins.perf_max = 2
            dn2f = s3p.tile([128, GROUP], BF16, tag="dn2f")
            nc.vector.tensor_scalar(dn2f[:], dn2e[:], EPS / (1.0 - NEG), None,
                                    ALU.add)
            s2 = s3p.tile([128, GROUP], BF16, tag="s2")
            nc.gpsimd.tensor_tensor(s2[:], dotm[:], dn2f[:], ALU.divide)

            mi3 = s3p.tile([128, 3, GROUP], BF16, tag="mi3")
            for i in range(3):
                nc.gpsimd.tensor_tensor(mi3[:, i, :], s2[:], dsb[:, i, :],
                                        ALU.mult)
            ot = outp.tile([128, 3, GROUP], BF16, tag="ot")
            nc.gpsimd.tensor_tensor(ot[:], xsb[:], mi3[:], ALU.subtract)

            c0 = u * SUPER
            nc.sync.dma_start(OUT[:, :, c0:c0 + GROUP], ot[0:64])
            nc.sync.dma_start(OUT[:, :, c0 + GROUP:c0 + SUPER], ot[64:128])

    nc.compile()
    return nc


_NC = None


def _get_nc():
    global _NC
    if _NC is None:
        _NC = _build_nc()
    return _NC


def _weight_stack(Wa, Wb, Wc, Wd):
    Z = np.zeros((64, 64), np.float32)
    WaT = Wa.T.astype(np.float32)
    WbT = Wb.T.astype(np.float32)
    W2nT = (Wa - Wc).T.astype(np.float32)
    W2T = (Wc - Wa).T.astype(np.float32)
    WdT = Wd.T.astype(np.float32)

    def vs(a, b):
        return np.vstack([a, b]).astype(np.float32)   # [128, 64]

    s0 = np.hstack([vs(WaT, W2nT), vs(WaT, W2T)])     # [128,128]
    s1 = np.block([[WbT, Z], [Z, WbT]]).astype(np.float32)
    s2 = np.hstack([vs(WaT, -WbT), np.zeros((128, 64), np.float32)])
    s3 = np.block([[WdT, Z], [Z, WdT]]).astype(np.float32)
    s4 = np.eye(128, dtype=np.float32)
    return np.ascontiguousarray(np.stack([s0, s1, s2, s3, s4]), np.float32)


def _prep_input(A):
    """[C, E, 3] f32 -> [NSUP, 128, NCHUNK*192] bf16, [s, c, e]-ordered per
    point so every field slice is packed, with each partition's 8 point-rows
    contiguous in HBM (3072B DMA descriptors)."""
    import ml_dtypes
    Ap = A.reshape(NSUP, NCHUNK, 128, E, 3).transpose(0, 2, 1, 4, 3)
    Ap = np.ascontiguousarray(Ap.reshape(NSUP, 128, NCHUNK * 192))
    return Ap.astype(ml_dtypes.bfloat16)


def run_full(X, J, Wa, Wb, Wc, Wd, trace=False, trace_kwargs=None):
    nc = _get_nc()
    wmm = _weight_stack(Wa, Wb, Wc, Wd)
    in_maps = []
    for b in range(B):
        in_maps.append({
            "XS": _prep_input(np.asarray(X[b], np.float32)),
            "JS": _prep_input(np.asarray(J[b], np.float32)),
            "WMM": wmm,
        })
    res = bass_utils.run_bass_kernel_spmd(
        nc, in_maps, core_ids=list(range(B)), trace=trace,
        **(trace_kwargs or {}))
    out = np.stack([np.asarray(res.results[b]["OUT"]).astype(np.float32)
                    for b in range(B)])
    return out, res


def kernel(X, J, Wa, Wb, Wc, Wd):
    out, _ = run_full(X, J, Wa, Wb, Wc, Wd)
    return out


# revision 19
# speedup vs baseline: 1.4249x; 1.0295x over previous
"""Trainium2 Bass kernel for nn_Complex_Only_46308337385506 (gnn_message_passing).

Math (derived + numerically validated against the jax reference):
  The per-edge orthonormal basis R (rows nU, nV, nJ) enters the output only
  through two per-edge scalars:
      gam = Jz*t,  t = 1/|J|
      a1  = sqrt(Jx^2+Jy^2)*t * sign(Jz+eps)
  With wt = gam*Xz - a1*Xx:
      Y0 = Wa@Xx + (Wa-Wc)@(a1*wt) + Wb@(gam*Xy)
      Y1 = Wa@Xy - Wb@(a1*Xz + gam*Xx)
      Y2 = Wa@Xz + (Wc-Wa)@(gam*wt) + Wb@(a1*Xy)
  followed by the VN leaky-relu stage:
      d = Wd@Y (over channel dim), dot = <Y,d>_3, dn2 = <d,d>_3
      out = Y - 0.8*min(dot,0)/(dn2+eps) * d

Sharding: data-parallel over batch B=8 -> one batch per NeuronCore.

Perf design (cost-model ~186us baseline -> target ~130us):
  - bf16 everywhere the 2e-2 tolerance allows (validated 6.2e-3 norm-rel in
    numpy): inputs are cast to bf16 host-side AND row-permuted so each
    partition's 8 point-rows are contiguous in HBM (3072B descriptors =
    full-speed DMA); output is written bf16 (1024B descriptors) and upcast
    host-side. DMA/super drops 6552ns -> 3276ns.
  - bf16 products/transposes: PE transposes cost 1 cyc/col, PSUM->SBUF
    copies hit the DVE 2x_1p mode (401ns vs 667 for f32).
  - prod slots are pre-paired (Xx,p2 | Xz,p3 | xyb,c8 | p4,p5) so each
    K=128 stationary combines two weight blocks; Y lands directly in
    xsb-layout [128=(g,f), 3, 512] PSUM via M=64 matmuls at partition
    offset 64g (5 matmuls/group), letting ONE wide ACT copy produce xsb.
  - VN scale uses TT-divide on Pool (no reciprocal+STT chain).
  - Op placement balances DVE (bf16 2x/4x ops, customs), ACT (sqrt/sign,
    wide f32 PSUM copies), Pool (flat-rate mixed-dtype muls, divide).
"""

import numpy as np
from contextlib import ExitStack

import concourse.bass as bass
import concourse.bacc as bacc
import concourse.tile as tile
from concourse import mybir
from concourse import bass_utils

F32 = mybir.dt.float32
BF16 = mybir.dt.bfloat16
AF = mybir.ActivationFunctionType
ALU = mybir.AluOpType

EPS = 1e-6
NEG = 0.2

B, C, E = 8, 16384, 64
SUPER = 1024           # points per super-iteration
NSUP = C // SUPER      # 16
GROUP = 512            # matmul free dim (points)
NCHUNK = 8             # 128-pt chunks per super


_CUSTOM_OPS = {}


def _register_custom_dve_ops():
    """Fused DVE ops (module-level, idempotent):
      SQSUM_ANT:  out = Src0^2 + Src1^2
      ADDSQ_ANT:  out = Src0 + Src1^2
      ADDSQS_ANT: out = (Src0 + Src1^2) * s0
    """
    if _CUSTOM_OPS:
        return _CUSTOM_OPS
    import numpy as _np
    from concourse import dve_ops
    from concourse.dve_spec import Spec, Src0, Src1, lower, sq, _has_src1

    def make(name, body, ref):
        spec = Spec(body=body, reference=ref)
        opcode = dve_ops._CUSTOM_DVE_ROW_BASE + len(dve_ops.OPS)
        shas = {}
        from concourse.dve_uop import DveOpSpec
        from concourse.dve_table_gen import dve_ver_for
        for ver in ("v3", "v4"):
            try:
                s = DveOpSpec(name=name, opcode=opcode,
                              uops=lower(spec, ver=ver),
                              rd1_en=_has_src1(spec))
                shas[ver] = s.sha(ver)
            except Exception:
                pass
        op = dve_ops.DveOp(name, spec, subdim=False, uops_sha=shas)
        dve_ops.OPS.append(op)
        dve_ops.CUSTOM_DVE_SPECS[name] = spec
        dve_ops._SUB_OPCODE_FOR_NAME[name] = opcode
        assert opcode < 0x20
        return op

    def fl(a):
        # operand APs may lower with different (collapsed vs not) free-dim
        # shapes; flatten to [p, -1] (same row-major order) before combining
        return _np.asarray(a).reshape(_np.asarray(a).shape[0], -1)

    _CUSTOM_OPS["SQSUM"] = make(
        "SQSUM_ANT", sq(Src0) + sq(Src1),
        lambda in0, in1, s0, s1, imm2:
            (fl(in0).astype(_np.float32) ** 2 + fl(in1).astype(_np.float32) ** 2))
    _CUSTOM_OPS["ADDSQ"] = make(
        "ADDSQ_ANT", Src0 + sq(Src1),
        lambda in0, in1, s0, s1, imm2:
            fl(in0).astype(_np.float32) + fl(in1).astype(_np.float32) ** 2)
    from concourse.dve_spec import C0
    _CUSTOM_OPS["ADDSQS"] = make(
        "ADDSQS_ANT", (Src0 + sq(Src1)) * C0,
        lambda in0, in1, s0, s1, imm2:
            (fl(in0).astype(_np.float32) + fl(in1).astype(_np.float32) ** 2) * s0)
    return _CUSTOM_OPS


def _pin_act_table_set(arch: str):
    """Steer the ACT table-set chooser: all funcs this kernel uses must
    first-match sqrt_and_others, so exactly one table load is emitted."""
    from concourse import hw_specs
    tables = hw_specs.get_activation_tables(arch)  # cached dict, mutate in place
    mine = {AF.Sqrt, AF.Sign, AF.Copy, AF.Identity, AF.Square}
    for name, funcs in tables.items():
        if name != "sqrt_and_others":
            funcs -= mine


def _build_nc():
    global OPS
    OPS = _register_custom_dve_ops()
    nc = bacc.Bacc("TRN2", debug=False)
    _pin_act_table_set(nc.m.arch)

    XS = nc.dram_tensor("XS", [NSUP, 128, NCHUNK * 192], BF16,
                        kind="ExternalInput").ap()
    JS = nc.dram_tensor("JS", [NSUP, 128, NCHUNK * 192], BF16,
                        kind="ExternalInput").ap()
    WMM = nc.dram_tensor("WMM", [5, 128, 128], F32, kind="ExternalInput").ap()
    OUT = nc.dram_tensor("OUT", [64, 3, C], BF16, kind="ExternalOutput").ap()

    with tile.TileContext(nc) as tc, ExitStack() as ctx:
        const = ctx.enter_context(tc.tile_pool(name="const", bufs=1))
        io = ctx.enter_context(tc.tile_pool(name="io", bufs=2))
        sa = ctx.enter_context(tc.tile_pool(name="sa", bufs=2))
        prodp = ctx.enter_context(tc.tile_pool(name="prodp", bufs=2))
        rhsp = ctx.enter_context(tc.tile_pool(name="rhsp", bufs=2))
        s3p = ctx.enter_context(tc.tile_pool(name="s3p", bufs=2))
        outp = ctx.enter_context(tc.tile_pool(name="outp", bufs=2))
        psT = ctx.enter_context(tc.tile_pool(name="psT", bufs=1, space="PSUM"))
        psY = ctx.enter_context(tc.tile_pool(name="psY", bufs=1, space="PSUM"))
        psD = ctx.enter_context(tc.tile_pool(name="psD", bufs=1, space="PSUM"))

        eps_c = const.tile([128, 1], F32, tag="eps_c")
        nc.gpsimd.memset(eps_c[:], EPS)

        # weights: f32 load once, cast to bf16 (one ACT pass)
        wsb = const.tile([128, 5, 128], F32)
        nc.sync.dma_start(wsb[:], WMM.rearrange("n p m -> p n m"))
        wbf = const.tile([128, 5, 128], BF16)
        nc.scalar.activation(wbf[:], wsb[:], AF.Copy)
        W0 = wbf[:, 0, :]      # [WaT;W2nT | WaT;W2T]  (cols 0:64 -> Y0, 64:128 -> Y2)
        W1 = wbf[:, 1, :]      # blkdiag(WbT, WbT)     (cols 0:64 -> Y0, 64:128 -> Y2)
        W2s = wbf[:, 2, 0:64]  # [WaT; -WbT]           (-> Y1)
        W3 = wbf[:, 3, :]      # blkdiag(WdT, WdT)
        IDb = wbf[:, 4, :]     # identity (bf16 transposes)

        for u in range(NSUP):
            xst = io.tile([128, NCHUNK * 192], BF16, tag="xs")
            jst = io.tile([128, NCHUNK * 192], BF16, tag="js")
            nc.sync.dma_start(xst[:], XS[u])
            nc.sync.dma_start(jst[:], JS[u])
            # host layout is [s, c, e]: every field slice is packed (innermost e)
            xv = xst[:].rearrange("p (s c e) -> p s c e", s=NCHUNK, c=3, e=E)
            jv = jst[:].rearrange("p (s c e) -> p s c e", s=NCHUNK, c=3, e=E)

            def v3(t):  # [128, 512] tile -> [128, 8, 64] view
                return t[:].rearrange("p (s e) -> p s e", s=NCHUNK, e=E)

            # ---- stage A: per-edge scalars gam, a1 ---------------------
            qn2 = sa.tile([128, 2, SUPER // 2], BF16, tag="qn2")
            q = qn2[:, 0, :]
            n2 = qn2[:, 1, :]
            nc.vector._custom_dve(OPS["SQSUM"], out=q,
                                  in0=jv[:, :, 0, :], in1=jv[:, :, 1, :]
                                  ).ins.perf_max = 2
            nc.vector._custom_dve(OPS["ADDSQ"], out=n2,
                                  in0=q, in1=jv[:, :, 2, :]
                                  ).ins.perf_max = 2
            sqs = sa.tile([128, 2, SUPER // 2], BF16, tag="sqs")
            nc.scalar.activation(sqs[:], qn2[:], AF.Sqrt)
            sq_ = sqs[:, 0, :]
            s_ = sqs[:, 1, :]
            sgn = sa.tile([128, SUPER // 2], BF16, tag="sgn")
            nc.scalar.activation(v3(sgn), jv[:, :, 2, :], AF.Sign, bias=eps_c[:])
            gam = sa.tile([128, SUPER // 2], BF16, tag="gam")
            nc.gpsimd.tensor_tensor(v3(gam), jv[:, :, 2, :],
                                    s_.rearrange("p (s e) -> p s e", s=NCHUNK, e=E),
                                    ALU.divide)
            a1u = sa.tile([128, SUPER // 2], BF16, tag="a1u")
            nc.gpsimd.tensor_tensor(a1u[:], sq_, s_, ALU.divide)
            a1 = sa.tile([128, SUPER // 2], BF16, tag="a1")
            nc.vector.tensor_tensor(a1[:], a1u[:], sgn[:], ALU.mult)

            # ---- stage B: products into paired prod slots --------------
            # slots: 0=Xx 1=p2 2=xyb 3=c8 4=Xz 5=p3 6=p4 7=p5
            prod = prodp.tile([128, NCHUNK, 8, E], BF16, tag="prod")
            nc.vector.tensor_copy(prod[:, :, 0:5:2, :], xv[:])
            m1 = sa.tile([128, SUPER // 2], BF16, tag="m1")
            m2 = sa.tile([128, SUPER // 2], BF16, tag="m2")
            m3 = sa.tile([128, SUPER // 2], BF16, tag="m3")
            m4 = sa.tile([128, SUPER // 2], BF16, tag="m4")
            nc.gpsimd.tensor_tensor(v3(m1), v3(gam), prod[:, :, 4, :], ALU.mult)
            nc.gpsimd.tensor_tensor(v3(m2), v3(a1), prod[:, :, 0, :], ALU.mult)
            nc.gpsimd.tensor_tensor(v3(m3), v3(a1), prod[:, :, 4, :], ALU.mult)
            nc.gpsimd.tensor_tensor(v3(m4), v3(gam), prod[:, :, 0, :], ALU.mult)
            wt = sa.tile([128, SUPER // 2], BF16, tag="wt")
            nc.vector.tensor_tensor(wt[:], m1[:], m2[:], ALU.subtract)
            nc.vector.tensor_tensor(prod[:, :, 3, :], v3(m3), v3(m4), ALU.add)
            nc.vector.tensor_tensor(prod[:, :, 1, :], v3(a1), v3(wt), ALU.mult)
            nc.vector.tensor_tensor(prod[:, :, 5, :], v3(gam), v3(wt), ALU.mult)
            nc.gpsimd.tensor_tensor(prod[:, :, 6, :], v3(gam), prod[:, :, 2, :],
                                    ALU.mult)
            nc.gpsimd.tensor_tensor(prod[:, :, 7, :], v3(a1), prod[:, :, 2, :],
                                    ALU.mult)

            # ---- per group: transpose, rh copies, Y matmuls ------------
            pY = psY.tile([128, 3, GROUP], F32, tag="pY")
            for g in range(2):
                tpA = psT.tile([128, 2, GROUP], BF16, tag="tpA")
                tpB = psT.tile([128, 2, GROUP], BF16, tag="tpB")
                for k in range(4):
                    s = 4 * g + k
                    sl = slice(128 * k, 128 * (k + 1))
                    nc.tensor.transpose(tpA[:, 0, sl], prod[:, s, 0:2, :], IDb)
                    nc.tensor.transpose(tpA[:, 1, sl], prod[:, s, 4:6, :], IDb)
                    nc.tensor.transpose(tpB[:, 0, sl], prod[:, s, 2:4, :], IDb)
                    nc.tensor.transpose(tpB[:, 1, sl], prod[:, s, 6:8, :], IDb)
                rhA = rhsp.tile([128, 2, GROUP], BF16, tag="rhA")
                rhB = rhsp.tile([128, 2, GROUP], BF16, tag="rhB")
                if g == 0:
                    nc.vector.tensor_copy(rhA[:], tpA[:])
                    nc.vector.tensor_copy(rhB[:], tpB[:])
                else:
                    nc.scalar.activation(rhA[:], tpA[:], AF.Copy)
                    nc.scalar.activation(rhB[:], tpB[:], AF.Copy)

                ro = slice(64 * g, 64 * (g + 1))
                rh1 = rhA[:, 0, :]   # [Xx; p2]
                rh2 = rhA[:, 1, :]   # [Xz; p3]
                rh3 = rhB[:, 0, :]   # [xyb; c8]
                rh4 = rhB[:, 1, :]   # [p4; p5]
                nc.tensor.matmul(pY[ro, 0, :], W0[:, 0:64], rh1,
                                 start=True, stop=False)
                nc.tensor.matmul(pY[ro, 0, :], W1[:, 0:64], rh4,
                                 start=False, stop=True)
                nc.tensor.matmul(pY[ro, 2, :], W0[:, 64:128], rh2,
                                 start=True, stop=False)
                nc.tensor.matmul(pY[ro, 2, :], W1[:, 64:128], rh4,
                                 start=False, stop=True)
                nc.tensor.matmul(pY[ro, 1, :], W2s, rh3, start=True, stop=True)

            # ---- Wd stage ----------------------------------------------
            xsb = s3p.tile([128, 3, GROUP], BF16, tag="xsb")
            nc.scalar.activation(xsb[:], pY[:], AF.Copy)
            pd = psD.tile([128, 3, GROUP], F32, tag="pd")
            for i in range(3):
                nc.tensor.matmul(pd[:, i, :], W3, xsb[:, i, :], start=True,
                                 stop=True)
            dsb = s3p.tile([128, 3, GROUP], BF16, tag="dsb")
            nc.scalar.activation(dsb[:], pd[:], AF.Copy)

            # ---- VN leaky relu -----------------------------------------
            P = s3p.tile([128, 3, GROUP], BF16, tag="P")
            nc.vector.tensor_tensor(P[:], xsb[:], dsb[:], ALU.mult)
            dot = s3p.tile([128, GROUP], BF16, tag="dot")
            nc.vector.tensor_tensor(dot[:], P[:, 0, :], P[:, 1, :], ALU.add)
            dot2 = s3p.tile([128, GROUP], BF16, tag="dot2")
            nc.vector.tensor_tensor(dot2[:], dot[:], P[:, 2, :], ALU.add)
            dotm = s3p.tile([128, GROUP], BF16, tag="dotm")
            nc.vector.tensor_scalar(dotm[:], dot2[:], 0.0, None, ALU.min)

            dn2 = s3p.tile([128, GROUP], BF16, tag="dn2")
            nc.vector._custom_dve(OPS["SQSUM"], out=dn2[:],
                                  in0=dsb[:, 0, :], in1=dsb[:, 1, :]
                                  ).ins.perf_max = 2
            dn2e = s3p.tile([128, GROUP], BF16, tag="dn2e")
            nc.vector._custom_dve(OPS["ADDSQS"], out=dn2e[:],
                                  in0=dn2[:], in1=dsb[:, 2, :],
                                  s0=1.0 / (1.0 - NEG)).ins.perf_max = 2
            dn2f = s3p.tile([128, GROUP], BF16, tag="dn2f")
            nc.vector.tensor_scalar(dn2f[:], dn2e[:], EPS / (1.0 - NEG), None,
                                    ALU.add)
            s2 = s3p.tile([128, GROUP], BF16, tag="s2")
            nc.gpsimd.tensor_tensor(s2[:], dotm[:], dn2f[:], ALU.divide)

            mi3 = s3p.tile([128, 3, GROUP], BF16, tag="mi3")
            for i in range(3):
                nc.gpsimd.tensor_tensor(mi3[:, i, :], s2[:], dsb[:, i, :],
                                        ALU.mult)
            ot = outp.tile([128, 3, GROUP], BF16, tag="ot")
            nc.gpsimd.tensor_tensor(ot[:], xsb[:], mi3[:], ALU.subtract)

            c0 = u * SUPER
            nc.sync.dma_start(OUT[:, :, c0:c0 + GROUP], ot[0:64])
            nc.sync.dma_start(OUT[:, :, c0 + GROUP:c0 + SUPER], ot[64:128])

    nc.compile()
    return nc


_NC = None


def _get_nc():
    global _NC
    if _NC is None:
        _NC = _build_nc()
    return _NC


def _weight_stack(Wa, Wb, Wc, Wd):
    Z = np.zeros((64, 64), np.float32)
    WaT = Wa.T.astype(np.float32)
    WbT = Wb.T.astype(np.float32)
    W2nT = (Wa - Wc).T.astype(np.float32)
    W2T = (Wc - Wa).T.astype(np.float32)
    WdT = Wd.T.astype(np.float32)

    def vs(a, b):
        return np.vstack([a, b]).astype(np.float32)   # [128, 64]

    s0 = np.hstack([vs(WaT, W2nT), vs(WaT, W2T)])     # [128,128]
    s1 = np.block([[WbT, Z], [Z, WbT]]).astype(np.float32)
    s2 = np.hstack([vs(WaT, -WbT), np.zeros((128, 64), np.float32)])
    s3 = np.block([[WdT, Z], [Z, WdT]]).astype(np.float32)
    s4 = np.eye(128, dtype=np.float32)
    return np.ascontiguousarray(np.stack([s0, s1, s2, s3, s4]), np.float32)


def _prep_input(A):
    """[C, E, 3] f32 -> [NSUP, 128, NCHUNK*192] bf16, [s, c, e]-ordered per
    point so every field slice is packed, with each partition's 8 point-rows
    contiguous in HBM (3072B DMA descriptors)."""
    import ml_dtypes
    Ap = A.reshape(NSUP, NCHUNK, 128, E, 3).transpose(0, 2, 1, 4, 3)
    Ap = np.ascontiguousarray(Ap.reshape(NSUP, 128, NCHUNK * 192))
    return Ap.astype(ml_dtypes.bfloat16)


def run_full(X, J, Wa, Wb, Wc, Wd, trace=False, trace_kwargs=None):
    nc = _get_nc()
    wmm = _weight_stack(Wa, Wb, Wc, Wd)
    in_maps = []
    for b in range(B):
        in_maps.append({
            "XS": _prep_input(np.asarray(X[b], np.float32)),
            "JS": _prep_input(np.asarray(J[b], np.float32)),
            "WMM": wmm,
        })
    res = bass_utils.run_bass_kernel_spmd(
        nc, in_maps, core_ids=list(range(B)), trace=trace,
        **(trace_kwargs or {}))
    out = np.stack([np.asarray(res.results[b]["OUT"]).astype(np.float32)
                    for b in range(B)])
    return out, res


def kernel(X, J, Wa, Wb, Wc, Wd):
    out, _ = run_full(X, J, Wa, Wb, Wc, Wd)
    return out
